# revision 67
# baseline (speedup 1.0000x reference)
"""Trainium2 Bass kernel for the BQNN boson-sampling MZI circuit (raw Bass).

Per sample: 6x6 unitary from 14 MZI Givens blocks applied to e0,e3 -> u,v;
out = |normalize(amp)|, amp_ab = u_a v_b + u_b v_a over 15 pairs.

Host-folded structure: const steps 0-3 -> constant real u0,v0; V1(+C1) on
those constants collapses to short zero-pruned linear chains over per-block
trig features; V2 is a packed generic layer; C2 emits straight into the
amp-stage layout.  sin/cos computed via quarter-angle double-angle (ACT Sin
is only valid on [-pi,pi]).  Raw Bass + explicit semaphore scoreboard
(TileContext tail-drain is rejected by this walrus build).

Layout: per core 32768 samples = 128 partitions x (n_chunks x F) free.
"""

import contextlib
import numpy as np

P = 128
NCORES = 8
BATCH = 262144
COREB = BATCH // NCORES        # 32768
FTOT = COREB // P              # 256

MODES = [[0, 1], [4, 5], [1, 2], [3, 4]] + [[0, 1], [2, 3], [4, 5], [1, 2], [3, 4]] * 2
OUT_PAIRS = [(i, j) for i in range(6) for j in range(i + 1, 6)]
DPAIRS = [(a, a + d) for d in range(1, 6) for a in range(6 - d)]
EPS = 1e-12

_CACHE = {}


def _host_consts(param_phi, param_theta):
    th = np.asarray(param_theta, np.float64)
    ph = np.asarray(param_phi, np.float64)
    U = np.eye(6, dtype=np.complex128)
    for k in range(4):
        i, j = MODES[k]
        c, s = np.cos(th[k]), np.sin(th[k])
        ri, rj = U[i, :].copy(), U[j, :].copy()
        U[i, :] = c * ri - s * rj
        U[j, :] = s * ri + c * rj
    u0, v0 = U[:, 0].copy(), U[:, 3].copy()
    c1 = [(MODES[7], th[4], ph[0]), (MODES[8], th[5], ph[1])]
    c2 = [(MODES[12], th[6], ph[2]), (MODES[13], th[7], ph[3])]
    return u0, v0, c1, c2


def _v1c1_exprs(u0, v0, c1):
    exprs = {}
    for w, w0 in ((0, u0), (1, v0)):
        for b in range(3):
            at, ab = w0[2 * b], w0[2 * b + 1]
            E, F_, G, H = f"E{b}", f"F{b}", f"G{b}", f"H{b}"
            CT, ST = f"CT1{b}", f"ST1{b}"
            exprs[(w, 2 * b, 0)] = {E: at.real, F_: -at.imag, ST: -ab.real}
            exprs[(w, 2 * b, 1)] = {E: at.imag, F_: at.real, ST: -ab.imag}
            exprs[(w, 2 * b + 1, 0)] = {G: at.real, H: -at.imag, CT: ab.real}
            exprs[(w, 2 * b + 1, 1)] = {G: at.imag, H: at.real, CT: ab.imag}

    def comb(*terms):
        out = {}
        for coef, d in terms:
            for k, v in d.items():
                out[k] = out.get(k, 0.0) + coef * v
        return out

    for (i, j), t, p in c1:
        al = np.exp(1j * p) * np.cos(t)
        be = np.exp(1j * p) * np.sin(t)
        c, s = np.cos(t), np.sin(t)
        for w in (0, 1):
            zir, zii = exprs[(w, i, 0)], exprs[(w, i, 1)]
            zjr, zji = exprs[(w, j, 0)], exprs[(w, j, 1)]
            exprs[(w, i, 0)] = comb((al.real, zir), (-al.imag, zii), (-s, zjr))
            exprs[(w, i, 1)] = comb((al.real, zii), (al.imag, zir), (-s, zji))
            exprs[(w, j, 0)] = comb((be.real, zir), (-be.imag, zii), (c, zjr))
            exprs[(w, j, 1)] = comb((be.real, zii), (be.imag, zir), (c, zji))
    return {k: {n: c for n, c in d.items() if abs(c) > 1e-30} for k, d in exprs.items()}


class Sched:
    """Per-engine in-order op lists + semaphore scoreboard.

    Engines: vector / scalar / gpsimd / sync.  Each op incs a sem space; DMA
    ops (engine sync) inc per-chunk-parity spaces so out-of-order DMA-queue
    completion can't satisfy another chunk's wait.
    """

    def __init__(self):
        self.ops = {"vector": [], "scalar": [], "gpsimd": [], "sync": []}
        self.counts = {}          # sem space -> current value
        self.waited = {"vector": {}, "scalar": {}, "gpsimd": {}, "sync": {}}
        self.writers = {}         # tile key -> [(space, val)]
        self.readers = {}

    def add(self, engine, fn, reads=(), writes=(), space=None, inc=1):
        space = space or engine
        self.counts.setdefault(space, 0)
        need = {}
        for r in list(reads) + list(writes):
            for ps, v in self.writers.get(r, ()):
                if ps != space:
                    need[ps] = max(need.get(ps, 0), v)
        for wkey in writes:
            for ps, v in self.readers.get(wkey, ()):
                if ps != space:
                    need[ps] = max(need.get(ps, 0), v)
        waits = []
        wt = self.waited[engine]
        for ps, v in sorted(need.items()):
            if wt.get(ps, 0) < v:
                waits.append((ps, v))
                wt[ps] = v
        after = self.counts[space] + inc
        self.counts[space] = after
        for r in reads:
            self.readers.setdefault(r, []).append((space, after))
        for wkey in writes:
            self.writers.setdefault(wkey, []).append((space, after))
        self.ops[engine].append((fn, waits, space, inc))


def _build(param_phi, param_theta, input_k, input_b, n_chunks=1):
    import concourse.bass as bass
    import concourse.mybir as mybir

    dt = mybir.dt
    f32 = dt.float32
    f16 = dt.float16
    AO = mybir.AluOpType
    AF = mybir.ActivationFunctionType

    F = FTOT // n_chunks
    u0, v0, c1, c2 = _host_consts(param_phi, param_theta)
    chains = _v1c1_exprs(u0, v0, c1)

    kv = np.asarray(input_k, np.float64)
    bv = np.asarray(input_b, np.float64)
    affine = not (np.allclose(kv, 1.0) and np.allclose(bv, 0.0))

    nc = bass.Bass()
    x_d = nc.dram_tensor("x", [COREB, 12], f16, kind="ExternalInput")
    o_d = nc.dram_tensor("out", [P, 15 * FTOT], f16, kind="ExternalOutput")
    if affine:
        kb_d = nc.dram_tensor("kb", [P, 24], f32, kind="ExternalInput")
    xv = x_d.rearrange("(p f) c -> p (f c)", p=P)
    ov = o_d

    ctx = contextlib.ExitStack()
    sb = lambda nm, w, dty=f32: ctx.enter_context(nc.sbuf_tensor(nm, [P, w], dty))
    npar = min(n_chunks, 2)
    tiles = []
    # fp16 tiles: all intermediate math (2x DVE tensor_tensor, 4x tensor_scalar)
    widths16 = dict(s4=12, c4=15, s2=15, sincm=15, coscm=12, st=24,
                    w_t=12, scrA=12, scrB=12, uvt=24, st2t=24)
    for par in range(npar):
        tl = {nm: sb(f"{nm}_{par}", w * F, f16) for nm, w in widths16.items()}
        tl["bufA"] = sb(f"bufA_{par}", 12 * F, f16)  # fp16 DMA-in x
        tl["sq32"] = sb(f"sq32_{par}", 30 * F)      # f32 squares (underflow-safe)
        tl["out32"] = sb(f"out32_{par}", 15 * F)    # f32 output staging
        tl["tot"] = sb(f"tot_{par}", F)
        tl["rr"] = sb(f"rr_{par}", F)
        tiles.append(tl)
    nbias = sb("nbias", 1)
    kb_t = sb("kbt", 24) if affine else None

    sched = Sched()
    S = sched.add

    if affine:
        S("sync", lambda: nc.sync.dma_start(kb_t[:, :], kb_d[:, :]),
          writes=["kb"], space="dma_in_0", inc=16)

    for ch in range(n_chunks):
        _emit_chunk(nc, sched, tiles[ch % npar], ch, ch % npar, F, xv, ov,
                    chains, c2, affine, kb_t, nbias, mybir)

    sems = {}
    with contextlib.ExitStack() as semctx:
        for space in sched.counts:
            sems[space] = semctx.enter_context(nc.semaphore(f"sem_{space}"))

        with nc.Block() as block:
            def runner(engine_name):
                def run(eng):
                    for fn, waits, space, inc in sched.ops[engine_name]:
                        for ps, v in waits:
                            eng.wait_ge(sems[ps], v)
                        inst = fn()
                        inst.then_inc(sems[space], inc)
                return run

            block.vector(runner("vector"))
            block.scalar(runner("scalar"))
            block.gpsimd(runner("gpsimd"))
            block.sync(runner("sync"))
        ctx.close()
    return nc


def _emit_chunk(nc, sched, tl, ch, par, F, xv, ov, chains, c2, affine, kb_t,
                nbias, mybir):
    dt = mybir.dt
    AO = mybir.AluOpType
    AF = mybir.ActivationFunctionType
    V, SC, G = nc.vector, nc.scalar, nc.gpsimd
    S = sched.add
    k = lambda name: f"{name}{par}"      # tile keys per buffer parity

    bufA, s4, c4, s2 = tl["bufA"], tl["s4"], tl["c4"], tl["s2"]
    sincm, coscm, st, w_t = tl["sincm"], tl["coscm"], tl["st"], tl["w_t"]
    scrA, scrB, uvt, tot, rr = tl["scrA"], tl["scrB"], tl["uvt"], tl["tot"], tl["rr"]
    st2t16, sq32, out32 = tl["st2t"], tl["sq32"], tl["out32"]
    xr = bufA

    # ---------- DMA in (two halves so trig overlaps the transfer)
    H = F // 2
    S("sync", lambda: nc.sync.dma_start(xr[:, 0:12 * H],
                                        xv[:, ch * 12 * F:ch * 12 * F + 12 * H]),
      writes=[k("bufA") + "h0"], space=f"dma_in_{par}", inc=16)
    S("sync", lambda: nc.sync.dma_start(xr[:, 12 * H:12 * F],
                                        xv[:, ch * 12 * F + 12 * H:(ch + 1) * 12 * F]),
      writes=[k("bufA") + "h1"], space=f"dma_in_{par}", inc=16)

    # ---------- xs affine (general path)
    trig_key = k("bufA")
    if affine:
        xs = scrA
        kbc = kb_t[:, 0:12].unsqueeze(1).broadcast_to([P, F, 12])
        bbc = kb_t[:, 12:24].unsqueeze(1).broadcast_to([P, F, 12])
        x3 = lambda t: t[:, 0:12 * F].rearrange("p (f c) -> p f c", c=12)
        S("vector", lambda: V.tensor_tensor(out=x3(xs), in0=x3(xr), in1=kbc, op=AO.mult),
          reads=[k("bufA") + "h0", k("bufA") + "h1", "kb"], writes=[k("scrA")])
        S("vector", lambda: V.tensor_tensor(out=x3(xs), in0=x3(xs), in1=bbc, op=AO.add),
          reads=["kb"], writes=[k("scrA")])
        trig_src, trig_key = xs, k("scrA")
    else:
        trig_src = xr

    # ---------- trig: s4=sin(x/4) [ACT], s2=sin(x/2) [ACT, |x|<=2pi safe];
    # cos = 1-2*s2^2 ; cos(x/2) = 1-2*s4^2 ; sin = 2*s2*cos(x/2). ACT 2 ops,
    # DVE 3 tensor_tensor (fp16 2x) + 3 tensor_scalar (fp16 4x).
    cm = lambda t, w=12: t[:, 0:w * F].rearrange("p (c f) -> p c f", c=w)
    for h in (0, 1):
        f0, f1 = h * H, (h + 1) * H
        scm = trig_src[:, 12 * f0:12 * f1].rearrange("p (f c) -> p c f", c=12)
        hk = trig_key + f"h{h}" if not affine else trig_key
        S("scalar", lambda scm=scm, f0=f0, f1=f1: SC.activation(
            cm(s4)[:, :, f0:f1], scm, AF.Sin, scale=0.25),
          reads=[hk], writes=[k("s4") + f"h{h}"])
        S("scalar", lambda scm=scm, f0=f0, f1=f1: SC.activation(
            cm(s2)[:, :, f0:f1], scm, AF.Sin, scale=0.5),
          reads=[hk], writes=[k("s2") + f"h{h}"])
    for h in (0, 1):
        f0, f1 = h * H, (h + 1) * H
        hs = lambda t, f0=f0, f1=f1: cm(t)[:, :, f0:f1]
        hk4, hk2 = k("s4") + f"h{h}", k("s2") + f"h{h}"
        hkc4 = k("c4") + f"h{h}"
        S("vector", lambda hs=hs: V.tensor_tensor(out=hs(c4), in0=hs(s4),
                                                  in1=hs(s4), op=AO.mult),
          reads=[hk4], writes=[hkc4])
        S("vector", lambda hs=hs: V.tensor_scalar(out=hs(c4), in0=hs(c4),
                                                  scalar1=-2.0, scalar2=1.0,
                                                  op0=AO.mult, op1=AO.add),
          reads=[hkc4], writes=[hkc4])
        S("vector", lambda hs=hs: V.tensor_tensor(out=hs(coscm), in0=hs(s2),
                                                  in1=hs(s2), op=AO.mult),
          reads=[hk2], writes=[k("coscm")])
        S("vector", lambda hs=hs: V.tensor_scalar(out=hs(coscm), in0=hs(coscm),
                                                  scalar1=-2.0, scalar2=1.0,
                                                  op0=AO.mult, op1=AO.add),
          reads=[k("coscm")], writes=[k("coscm")])
        S("vector", lambda hs=hs: V.tensor_tensor(out=hs(sincm), in0=hs(s2),
                                                  in1=hs(c4), op=AO.mult),
          reads=[hk2, hkc4], writes=[k("sincm")])
        S("vector", lambda hs=hs: V.tensor_scalar(out=hs(sincm), in0=hs(sincm),
                                                  scalar1=2.0, scalar2=None, op0=AO.mult),
          reads=[k("sincm")], writes=[k("sincm")])

    CP1, SP1 = coscm[:, 0:3 * F], sincm[:, 0:3 * F]
    CT1, ST1 = coscm[:, 3 * F:6 * F], sincm[:, 3 * F:6 * F]
    CP2, SP2 = coscm[:, 6 * F:9 * F], sincm[:, 6 * F:9 * F]
    CT2, ST2 = coscm[:, 9 * F:12 * F], sincm[:, 9 * F:12 * F]

    # ---------- features E,F,G,H -> s4 slot (dead after trig)
    efgh = s4
    for idx, (a, b) in enumerate(((CP1, CT1), (SP1, CT1), (CP1, ST1), (SP1, ST1))):
        S("vector", lambda a=a, b=b, idx=idx: V.tensor_tensor(
            out=efgh[:, idx * 3 * F:(idx + 1) * 3 * F], in0=a, in1=b, op=AO.mult),
          reads=[k("sincm"), k("coscm")], writes=[k("s4")])

    def feat_ap(name):
        base = {"E": 0, "F": 1, "G": 2, "H": 3}
        b = int(name[-1])
        if name[0] in base and len(name) == 2:
            i = base[name[0]] * 3 + b
            return efgh[:, i * F:(i + 1) * F]
        if name.startswith("CT1"):
            return coscm[:, (3 + b) * F:(4 + b) * F]
        if name.startswith("ST1"):
            return sincm[:, (3 + b) * F:(4 + b) * F]
        raise KeyError(name)

    def unit_ap(tile, w, m, comp):
        row = (0 if m % 2 == 0 else 2) + comp
        off = row * 6 * F + w * 3 * F + (m // 2) * F
        return tile[:, off:off + F]

    # ---------- V1+C1 chains -> st
    rkeys = [k("s4"), k("sincm"), k("coscm")]
    chain_items = sorted(chains.items(), key=lambda it: it[0][1] % 2)  # T rows first
    ukey = lambda w, m, comp: k("st") + f"x{w}{m}{comp}"
    stT_keys = [ukey(w, m, c_) for (w, m, c_) in chains if m % 2 == 0]
    stB_keys = [ukey(w, m, c_) for (w, m, c_) in chains if m % 2 == 1]
    # first terms / memsets batched first (ACT + gpsimd run ahead of DVE)
    for (w, m, comp), expr in chain_items:
        out_ap = unit_ap(st, w, m, comp)
        stk = ukey(w, m, comp)
        items = list(expr.items())
        if not items:
            S("gpsimd", lambda o=out_ap: G.memset(o, 0.0), writes=[stk])
        else:
            n0, c0 = items[0]
            S("scalar", lambda o=out_ap, n=n0, c=c0: SC.mul(
                o, feat_ap(n), float(c)),
              reads=rkeys, writes=[stk])
    for (w, m, comp), expr in chain_items:
        out_ap = unit_ap(st, w, m, comp)
        stk = ukey(w, m, comp)
        items = list(expr.items())
        # B-rows (m odd) are needed later (V2 rotation); route half to the
        # otherwise-idle Pool engine
        for n, c in items[1:]:
            S("vector", lambda o=out_ap, n=n, c=c: V.scalar_tensor_tensor(
                out=o, in0=feat_ap(n), scalar=float(c), in1=o,
                op0=AO.mult, op1=AO.add),
              reads=rkeys, writes=[stk])

    # ---------- V2 phase: W = ep2 * T
    bc2 = lambda t: t.unsqueeze(1).broadcast_to([P, 2, 3 * F])
    g2 = lambda ap: ap.rearrange("p (g q) -> p g q", g=2)
    TRE, TIM = st[:, 0:6 * F], st[:, 6 * F:12 * F]
    WRE, WIM = w_t[:, 0:6 * F], w_t[:, 6 * F:12 * F]
    vk = stT_keys + [k("coscm"), k("sincm")]
    S("vector", lambda: V.tensor_tensor(out=g2(WRE), in0=bc2(CP2), in1=g2(TRE), op=AO.mult),
      reads=vk, writes=[k("w_t")])
    S("vector", lambda: V.tensor_tensor(out=g2(scrB[:, 0:6 * F]), in0=bc2(SP2),
                                        in1=g2(TIM), op=AO.mult),
      reads=vk, writes=[k("scrB")])
    S("vector", lambda: V.tensor_tensor(out=WRE, in0=WRE, in1=scrB[:, 0:6 * F],
                                        op=AO.subtract),
      reads=[k("scrB")], writes=[k("w_t")])
    S("vector", lambda: V.tensor_tensor(out=g2(WIM), in0=bc2(CP2), in1=g2(TIM), op=AO.mult),
      reads=vk, writes=[k("w_t")])
    S("vector", lambda: V.tensor_tensor(out=g2(scrB[:, 6 * F:12 * F]), in0=bc2(SP2),
                                        in1=g2(TRE), op=AO.mult),
      reads=vk, writes=[k("scrB")])
    S("vector", lambda: V.tensor_tensor(out=WIM, in0=WIM, in1=scrB[:, 6 * F:12 * F],
                                        op=AO.add),
      reads=[k("scrB")], writes=[k("w_t")])

    # ---------- V2 rotation -> st2t (fp16)
    st2t = st2t16
    bc4 = lambda t: t.unsqueeze(1).broadcast_to([P, 4, 3 * F])
    g4 = lambda ap: ap.rearrange("p (g q) -> p g q", g=4)
    Brows = st[:, 12 * F:24 * F]
    Tp, Bp = st2t[:, 0:12 * F], st2t[:, 12 * F:24 * F]
    S("vector", lambda: V.tensor_tensor(out=g4(Tp), in0=bc4(CT2), in1=g4(w_t[:, :]),
                                        op=AO.mult),
      reads=[k("w_t"), k("coscm")], writes=[k("bufA")])
    S("vector", lambda: V.tensor_tensor(out=g4(scrB[:, :]), in0=bc4(ST2), in1=g4(Brows),
                                        op=AO.mult),
      reads=stB_keys + [k("sincm")], writes=[k("scrB")])
    S("vector", lambda: V.tensor_tensor(out=Tp, in0=Tp, in1=scrB[:, :], op=AO.subtract),
      reads=[k("scrB")], writes=[k("bufA")])
    S("vector", lambda: V.tensor_tensor(out=g4(Bp), in0=bc4(ST2), in1=g4(w_t[:, :]),
                                        op=AO.mult),
      reads=[k("w_t"), k("sincm")], writes=[k("bufA")])
    S("vector", lambda: V.tensor_tensor(out=g4(scrA[:, :]), in0=bc4(CT2), in1=g4(Brows),
                                        op=AO.mult),
      reads=stB_keys + [k("coscm")], writes=[k("scrA")])
    S("vector", lambda: V.tensor_tensor(out=Bp, in0=Bp, in1=scrA[:, :], op=AO.add),
      reads=[k("scrA")], writes=[k("bufA")])

    # ---------- C2 -> uvt = [URE|UIM|VRE|VIM]
    st2t_v = st2t[:, :].rearrange("p (r h b q) -> p r h b q", r=4, h=2, b=3)
    uvt_v = uvt[:, :].rearrange("p (h c b q) -> p h c b q", h=2, c=2, b=6)
    uv_src = lambda m, comp: st2t_v[:, (0 if m % 2 == 0 else 2) + comp, :, m // 2, :]
    uv_dst = lambda m, comp: uvt_v[:, :, comp, m, :]

    SC2 = float(np.sqrt(128.0))  # scale u,v by sqrt(128): amp x128 so fp16
    # squares stay normal; normalize cancels the scale exactly.
    c2_units = []
    for (i, j), t, p in c2:
        al = np.exp(1j * p) * np.cos(t) * SC2
        be = np.exp(1j * p) * np.sin(t) * SC2
        c, s = np.cos(t) * SC2, np.sin(t) * SC2
        for comp in (0, 1):
            sgn = -1.0 if comp == 0 else 1.0
            for (tgt, c0, c1_, c2_) in (
                (i, al.real, sgn * al.imag, -s),
                (j, be.real, sgn * be.imag, c),
            ):
                c2_units.append((i, j, comp, tgt, c0, c1_, c2_))
    # first terms batched on ACT (per-unit uvt keys -> no false serialization)
    for (i, j, comp, tgt, c0, c1_, c2_) in c2_units:
        uk = k("uvt") + f"u{tgt}c{comp}"
        S("scalar", lambda tgt=tgt, comp=comp, m=i, c0=c0: SC.mul(
            uv_dst(tgt, comp), uv_src(m, comp), float(c0)),
          reads=[k("bufA")], writes=[uk])
    for un, (i, j, comp, tgt, c0, c1_, c2_) in enumerate(c2_units):
        uk = k("uvt") + f"u{tgt}c{comp}"
        o = uv_dst(tgt, comp)
        S("vector", lambda o=o, m=i, cc=1 - comp, c1_=c1_: V.scalar_tensor_tensor(
            out=o, in0=uv_src(m, cc), scalar=float(c1_), in1=o,
            op0=AO.mult, op1=AO.add),
          reads=[k("bufA")], writes=[uk])
        S("vector", lambda o=o, m=j, cc=comp, c2_=c2_: V.scalar_tensor_tensor(
            out=o, in0=uv_src(m, cc), scalar=float(c2_), in1=o,
            op0=AO.mult, op1=AO.add),
          reads=[k("bufA")], writes=[uk])

    uvt_keys = [k("uvt")] + [k("uvt") + f"u{m}c{c_}" for m in (1, 2, 3, 4)
                             for c_ in (0, 1)]
    # modes 0,5 pass-through copies
    for w in (0, 1):
        for comp in (0, 1):
            src_off = comp * 6 * F + w * 3 * F
            d0 = uvt[:, w * 12 * F + comp * 6 * F:w * 12 * F + comp * 6 * F + F]
            d5 = uvt[:, w * 12 * F + comp * 6 * F + 5 * F:
                     w * 12 * F + comp * 6 * F + 6 * F]
            S("scalar", lambda d=d0, o=src_off: SC.mul(d, st2t[:, o:o + F], SC2),
              reads=[k("bufA")], writes=[k("uvt")])
            S("scalar", lambda d=d5, o=src_off: SC.mul(d, st2t[:, o + 14 * F:o + 15 * F], SC2),
              reads=[k("bufA")], writes=[k("uvt")])

    # ---------- amplitudes (d-major pair order)
    URE, UIM = uvt[:, 0:6 * F], uvt[:, 6 * F:12 * F]
    VRE, VIM = uvt[:, 12 * F:18 * F], uvt[:, 18 * F:24 * F]
    UU = uvt[:, 0:12 * F].rearrange("p (c q) -> p c q", c=2)
    VV = uvt[:, 12 * F:24 * F].rearrange("p (c q) -> p c q", c=2)
    amp_re, amp_im = c4, s2
    # two-pass amp: all products first (Pool t12 || DVE t34+im), then all
    # combines -- no in-order stall of DVE combines behind Pool products.
    # Scratch regions carved from dead fp16 tiles; safety: every product op
    # waits on uvt_keys, whose writers (C2, vector+scalar) postdate all prior
    # readers of these tiles on their engines.
    freelist = [[st, 0, 24], [st2t16, 0, 24], [scrA, 0, 12], [scrB, 0, 12],
                [w_t, 0, 12], [coscm, 0, 12], [s4, 0, 12], [sincm, 0, 15]]
    def alloc(nF):
        for ent in freelist:
            t, off, cap = ent
            if cap - off >= nF:
                ent[1] = off + nF
                return t[:, off * F:(off + nF) * F]
        raise RuntimeError("amp scratch oom")
    regs = {}
    for d in range(1, 6):
        w = (6 - d) * F
        wf = 6 - d
        regs[d] = dict(t12=alloc(2 * wf), t34=alloc(2 * wf),
                       im=[alloc(wf) for _ in range(4)])
    rk = lambda d, nm: k("ampr") + f"{d}{nm}"
    for d in range(1, 6):
        w = (6 - d) * F
        r = regs[d]
        t12v = r["t12"].rearrange("p (c q) -> p c q", c=2)
        t34v = r["t34"].rearrange("p (c q) -> p c q", c=2)
        S("gpsimd", lambda t12v=t12v, w=w, d=d: G.tensor_tensor(
            out=t12v, in0=UU[:, :, 0:w], in1=VV[:, :, d * F:d * F + w], op=AO.mult),
          reads=uvt_keys, writes=[rk(d, "t12")])
        S("vector", lambda t34v=t34v, w=w, d=d: V.tensor_tensor(
            out=t34v, in0=UU[:, :, d * F:d * F + w], in1=VV[:, :, 0:w], op=AO.mult),
          reads=uvt_keys, writes=[rk(d, "t34")])
        for ii, (ua, va, o1, o2) in enumerate((
                (URE, VIM, 0, 1), (UIM, VRE, 0, 1),
                (URE, VIM, 1, 0), (UIM, VRE, 1, 0))):
            en, enm = (G, "gpsimd") if ii == 0 else (V, "vector")
            S(enm, lambda w=w, d=d, im=r["im"][ii], ua=ua, va=va,
              o1=o1, o2=o2, e=en: e.tensor_tensor(
                out=im[:, 0:w], in0=ua[:, o1 * d * F:o1 * d * F + w],
                in1=va[:, o2 * d * F:o2 * d * F + w], op=AO.mult),
              reads=uvt_keys, writes=[rk(d, f"im{ii}")])
    doff = 0
    for d in range(1, 6):
        w = (6 - d) * F
        r = regs[d]
        are = amp_re[:, doff:doff + w]
        aim = amp_im[:, doff:doff + w]
        S("vector", lambda are=are, w=w, t12=r["t12"]: V.tensor_tensor(
            out=are, in0=t12[:, 0:w], in1=t12[:, w:2 * w], op=AO.subtract),
          reads=[rk(d, "t12")], writes=[k("c4")])
        S("vector", lambda are=are, w=w, t34=r["t34"]: V.tensor_tensor(
            out=are, in0=are, in1=t34[:, 0:w], op=AO.add),
          reads=[rk(d, "t34")], writes=[k("c4")])
        S("vector", lambda are=are, w=w, t34=r["t34"]: V.tensor_tensor(
            out=are, in0=are, in1=t34[:, w:2 * w], op=AO.subtract),
          reads=[rk(d, "t34")], writes=[k("c4")])
        S("vector", lambda aim=aim, w=w, i0=r["im"][0], i1=r["im"][1]:
          V.tensor_tensor(out=aim, in0=i0[:, 0:w], in1=i1[:, 0:w], op=AO.add),
          reads=[rk(d, "im0"), rk(d, "im1")], writes=[k("s2")])
        S("vector", lambda aim=aim, w=w, i2=r["im"][2]: V.tensor_tensor(
            out=aim, in0=aim, in1=i2[:, 0:w], op=AO.add),
          reads=[rk(d, "im2")], writes=[k("s2")])
        S("vector", lambda aim=aim, w=w, i3=r["im"][3]: V.tensor_tensor(
            out=aim, in0=aim, in1=i3[:, 0:w], op=AO.add),
          reads=[rk(d, "im3")], writes=[k("s2")])
        doff += w

    # ---------- tail (amp scaled x128 so fp16 squares stay normal-range;
    # normalize cancels the scale)
    sqre, sqim = sincm, st  # dead fp16 tiles, [15F] each
    S("vector", lambda: V.tensor_tensor(out=sqre[:, 0:15 * F], in0=amp_re[:, 0:15 * F],
                                        in1=amp_re[:, 0:15 * F], op=AO.mult),
      reads=[k("c4")], writes=[k("sqre"), k("sincm")])
    S("vector", lambda: V.tensor_tensor(out=sqim[:, 0:15 * F], in0=amp_im[:, 0:15 * F],
                                        in1=amp_im[:, 0:15 * F], op=AO.mult),
      reads=[k("s2")], writes=[k("sqim")] + stT_keys + stB_keys)
    S("vector", lambda: V.tensor_tensor(out=sqre[:, 0:15 * F], in0=sqre[:, 0:15 * F],
                                        in1=sqim[:, 0:15 * F], op=AO.add),
      reads=[k("sqre"), k("sqim")], writes=[k("sqre")])
    # fold q8..14 into q0..6 (fp16 2x), then reduce the remaining 8 q-slots
    S("vector", lambda: V.tensor_tensor(out=sqim[:, 0:7 * F], in0=sqre[:, 0:7 * F],
                                        in1=sqre[:, 8 * F:15 * F], op=AO.add),
      reads=[k("sqre")], writes=[k("sqim2")])
    S("vector", lambda: V.tensor_scalar(out=sqim[:, 7 * F:8 * F], in0=sqre[:, 14 * F:15 * F],
                                        scalar1=1.0, scalar2=None, op0=AO.mult),
      reads=[k("sqre")], writes=[k("sqim2")])
    S("vector", lambda: V.tensor_tensor(out=sqim[:, 0:4 * F], in0=sqim[:, 0:4 * F],
                                        in1=sqim[:, 4 * F:8 * F], op=AO.add),
      reads=[k("sqim2")], writes=[k("sqim2")])
    S("vector", lambda: V.tensor_tensor(out=sqim[:, 0:2 * F], in0=sqim[:, 0:2 * F],
                                        in1=sqim[:, 2 * F:4 * F], op=AO.add),
      reads=[k("sqim2")], writes=[k("sqim2")])
    S("vector", lambda: V.tensor_tensor(out=tot[:, :], in0=sqim[:, 0:F],
                                        in1=sqim[:, F:2 * F], op=AO.add),
      reads=[k("sqim2")], writes=[k("tot")])
    S("vector", lambda: V.tensor_scalar_max(out=tot[:, :], in0=tot[:, :],
                                            scalar1=float(EPS * EPS)),
      writes=[k("tot")])
    S("scalar", lambda: SC.activation(rr[:, :], tot[:, :], AF.Sqrt),
      reads=[k("tot")], writes=[k("rr")])
    S("vector", lambda: V.reciprocal(out=rr[:, :], in_=rr[:, :]), writes=[k("rr")])
    rr16 = w_t  # dead fp16 [12F]; use [0:F]
    S("vector", lambda: V.tensor_scalar(out=rr16[:, 0:F], in0=rr[:, 0:F],
                                        scalar1=1.0, scalar2=None, op0=AO.mult),
      reads=[k("rr")], writes=[k("rr16")])
    root = c4     # amp_re source; sqrt overwrites after squares done
    out16 = st2t16  # dead fp16 [24F]
    cmaj = lambda t: t[:, 0:15 * F].rearrange("p (q f) -> p q f", q=15)
    # split sqrt/scale/DMA by q-ranges: output DMA slices stay contiguous
    # (4KB runs) instead of 256B strided f-halves
    for h, (q0, q1) in enumerate(((0, 8), (8, 15))):
        S("scalar", lambda q0=q0, q1=q1: SC.activation(
            cmaj(root)[:, q0:q1, :], cmaj(sqre)[:, q0:q1, :], AF.Sqrt),
          reads=[k("sqre")], writes=[k("c4") + f"h{h}"])
        S("vector", lambda q0=q0, q1=q1: V.tensor_tensor(
            out=cmaj(out16)[:, q0:q1, :],
            in0=cmaj(root)[:, q0:q1, :],
            in1=rr16[:, 0:F].unsqueeze(1).broadcast_to([P, q1 - q0, F]),
            op=AO.mult),
          reads=[k("c4") + f"h{h}", k("rr16")],
          writes=[k("out16") + f"o{h}"] + stT_keys + stB_keys)
        S("sync", lambda q0=q0, q1=q1: nc.sync.dma_start(
            ov[:, 15 * F * ch + q0 * F:15 * F * ch + q1 * F],
            out16[:, q0 * F:q1 * F]),
          reads=[k("out16") + f"o{h}"], space=f"dma_out_{par}", inc=16)


def kernel(x, param_phi, param_theta, input_k, input_b):
    from concourse.bass_utils import run_bass_kernel_spmd

    x = np.ascontiguousarray(np.asarray(x, np.float16))
    key = (tuple(np.asarray(param_phi, np.float64).tolist()),
           tuple(np.asarray(param_theta, np.float64).tolist()),
           tuple(np.asarray(input_k, np.float64).tolist()),
           tuple(np.asarray(input_b, np.float64).tolist()))
    if key not in _CACHE:
        _CACHE[key] = _build(param_phi, param_theta, input_k, input_b)
    nc = _CACHE[key]

    kv = np.asarray(input_k, np.float64)
    bv = np.asarray(input_b, np.float64)
    affine = not (np.allclose(kv, 1.0) and np.allclose(bv, 0.0))

    in_maps = []
    for c in range(NCORES):
        m = {"x": x[c * COREB:(c + 1) * COREB]}
        if affine:
            kb = np.concatenate([kv, bv]).astype(np.float32)[None, :].repeat(P, 0)
            m["kb"] = np.ascontiguousarray(kb)
        in_maps.append(m)

    res = run_bass_kernel_spmd(nc, in_maps, core_ids=list(range(NCORES)))
    # device out: [P, 15*FTOT] fp16, layout [p, q*F + f] (q = d-major pair idx)
    devs = []
    for r in res.results:
        d = np.asarray(r["out"], np.float32).reshape(P, 15, FTOT)
        devs.append(d.transpose(0, 2, 1).reshape(COREB, 15))
    dev = np.concatenate(devs, axis=0)
    out = np.empty_like(dev)
    for dpos, pair in enumerate(DPAIRS):
        out[:, OUT_PAIRS.index(pair)] = dev[:, dpos]
    return out



# revision 68
# speedup vs baseline: 1.0026x; 1.0026x over previous
"""Trainium2 Bass kernel for the BQNN boson-sampling MZI circuit (raw Bass).

Per sample: 6x6 unitary from 14 MZI Givens blocks applied to e0,e3 -> u,v;
out = |normalize(amp)|, amp_ab = u_a v_b + u_b v_a over 15 pairs.

Host-folded structure: const steps 0-3 -> constant real u0,v0; V1(+C1) on
those constants collapses to short zero-pruned linear chains over per-block
trig features; V2 is a packed generic layer; C2 emits straight into the
amp-stage layout.  sin/cos computed via quarter-angle double-angle (ACT Sin
is only valid on [-pi,pi]).  Raw Bass + explicit semaphore scoreboard
(TileContext tail-drain is rejected by this walrus build).

Layout: per core 32768 samples = 128 partitions x (n_chunks x F) free.
"""

import contextlib
import numpy as np

P = 128
NCORES = 8
BATCH = 262144
COREB = BATCH // NCORES        # 32768
FTOT = COREB // P              # 256

MODES = [[0, 1], [4, 5], [1, 2], [3, 4]] + [[0, 1], [2, 3], [4, 5], [1, 2], [3, 4]] * 2
OUT_PAIRS = [(i, j) for i in range(6) for j in range(i + 1, 6)]
DPAIRS = [(a, a + d) for d in range(1, 6) for a in range(6 - d)]
EPS = 1e-12

_CACHE = {}


def _host_consts(param_phi, param_theta):
    th = np.asarray(param_theta, np.float64)
    ph = np.asarray(param_phi, np.float64)
    U = np.eye(6, dtype=np.complex128)
    for k in range(4):
        i, j = MODES[k]
        c, s = np.cos(th[k]), np.sin(th[k])
        ri, rj = U[i, :].copy(), U[j, :].copy()
        U[i, :] = c * ri - s * rj
        U[j, :] = s * ri + c * rj
    u0, v0 = U[:, 0].copy(), U[:, 3].copy()
    c1 = [(MODES[7], th[4], ph[0]), (MODES[8], th[5], ph[1])]
    c2 = [(MODES[12], th[6], ph[2]), (MODES[13], th[7], ph[3])]
    return u0, v0, c1, c2


def _v1c1_exprs(u0, v0, c1):
    exprs = {}
    for w, w0 in ((0, u0), (1, v0)):
        for b in range(3):
            at, ab = w0[2 * b], w0[2 * b + 1]
            E, F_, G, H = f"E{b}", f"F{b}", f"G{b}", f"H{b}"
            CT, ST = f"CT1{b}", f"ST1{b}"
            exprs[(w, 2 * b, 0)] = {E: at.real, F_: -at.imag, ST: -ab.real}
            exprs[(w, 2 * b, 1)] = {E: at.imag, F_: at.real, ST: -ab.imag}
            exprs[(w, 2 * b + 1, 0)] = {G: at.real, H: -at.imag, CT: ab.real}
            exprs[(w, 2 * b + 1, 1)] = {G: at.imag, H: at.real, CT: ab.imag}

    def comb(*terms):
        out = {}
        for coef, d in terms:
            for k, v in d.items():
                out[k] = out.get(k, 0.0) + coef * v
        return out

    for (i, j), t, p in c1:
        al = np.exp(1j * p) * np.cos(t)
        be = np.exp(1j * p) * np.sin(t)
        c, s = np.cos(t), np.sin(t)
        for w in (0, 1):
            zir, zii = exprs[(w, i, 0)], exprs[(w, i, 1)]
            zjr, zji = exprs[(w, j, 0)], exprs[(w, j, 1)]
            exprs[(w, i, 0)] = comb((al.real, zir), (-al.imag, zii), (-s, zjr))
            exprs[(w, i, 1)] = comb((al.real, zii), (al.imag, zir), (-s, zji))
            exprs[(w, j, 0)] = comb((be.real, zir), (-be.imag, zii), (c, zjr))
            exprs[(w, j, 1)] = comb((be.real, zii), (be.imag, zir), (c, zji))
    return {k: {n: c for n, c in d.items() if abs(c) > 1e-30} for k, d in exprs.items()}


class Sched:
    """Per-engine in-order op lists + semaphore scoreboard.

    Engines: vector / scalar / gpsimd / sync.  Each op incs a sem space; DMA
    ops (engine sync) inc per-chunk-parity spaces so out-of-order DMA-queue
    completion can't satisfy another chunk's wait.
    """

    def __init__(self):
        self.ops = {"vector": [], "scalar": [], "gpsimd": [], "sync": []}
        self.counts = {}          # sem space -> current value
        self.waited = {"vector": {}, "scalar": {}, "gpsimd": {}, "sync": {}}
        self.writers = {}         # tile key -> [(space, val)]
        self.readers = {}

    def add(self, engine, fn, reads=(), writes=(), space=None, inc=1):
        space = space or engine
        self.counts.setdefault(space, 0)
        need = {}
        for r in list(reads) + list(writes):
            for ps, v in self.writers.get(r, ()):
                if ps != space:
                    need[ps] = max(need.get(ps, 0), v)
        for wkey in writes:
            for ps, v in self.readers.get(wkey, ()):
                if ps != space:
                    need[ps] = max(need.get(ps, 0), v)
        waits = []
        wt = self.waited[engine]
        for ps, v in sorted(need.items()):
            if wt.get(ps, 0) < v:
                waits.append((ps, v))
                wt[ps] = v
        after = self.counts[space] + inc
        self.counts[space] = after
        for r in reads:
            self.readers.setdefault(r, []).append((space, after))
        for wkey in writes:
            self.writers.setdefault(wkey, []).append((space, after))
        self.ops[engine].append((fn, waits, space, inc))


def _build(param_phi, param_theta, input_k, input_b, n_chunks=1):
    import concourse.bass as bass
    import concourse.mybir as mybir

    dt = mybir.dt
    f32 = dt.float32
    f16 = dt.float16
    AO = mybir.AluOpType
    AF = mybir.ActivationFunctionType

    F = FTOT // n_chunks
    u0, v0, c1, c2 = _host_consts(param_phi, param_theta)
    chains = _v1c1_exprs(u0, v0, c1)

    kv = np.asarray(input_k, np.float64)
    bv = np.asarray(input_b, np.float64)
    affine = not (np.allclose(kv, 1.0) and np.allclose(bv, 0.0))

    nc = bass.Bass()
    x_d = nc.dram_tensor("x", [COREB, 12], f16, kind="ExternalInput")
    o_d = nc.dram_tensor("out", [P, 15 * FTOT], f16, kind="ExternalOutput")
    if affine:
        kb_d = nc.dram_tensor("kb", [P, 24], f32, kind="ExternalInput")
    xv = x_d.rearrange("(p f) c -> p (f c)", p=P)
    ov = o_d

    ctx = contextlib.ExitStack()
    sb = lambda nm, w, dty=f32: ctx.enter_context(nc.sbuf_tensor(nm, [P, w], dty))
    npar = min(n_chunks, 2)
    tiles = []
    # fp16 tiles: all intermediate math (2x DVE tensor_tensor, 4x tensor_scalar)
    widths16 = dict(s4=12, c4=15, s2=15, sincm=15, coscm=12, st=24,
                    w_t=12, scrA=12, scrB=12, uvt=24, st2t=24)
    for par in range(npar):
        tl = {nm: sb(f"{nm}_{par}", w * F, f16) for nm, w in widths16.items()}
        tl["bufA"] = sb(f"bufA_{par}", 12 * F, f16)  # fp16 DMA-in x
        tl["sq32"] = sb(f"sq32_{par}", 30 * F)      # f32 squares (underflow-safe)
        tl["out32"] = sb(f"out32_{par}", 15 * F)    # f32 output staging
        tl["tot"] = sb(f"tot_{par}", F)
        tl["rr"] = sb(f"rr_{par}", F)
        tiles.append(tl)
    nbias = sb("nbias", 1)
    kb_t = sb("kbt", 24) if affine else None

    sched = Sched()
    S = sched.add

    if affine:
        S("sync", lambda: nc.sync.dma_start(kb_t[:, :], kb_d[:, :]),
          writes=["kb"], space="dma_in_0", inc=16)

    for ch in range(n_chunks):
        _emit_chunk(nc, sched, tiles[ch % npar], ch, ch % npar, F, xv, ov,
                    chains, c2, affine, kb_t, nbias, mybir)

    sems = {}
    with contextlib.ExitStack() as semctx:
        for space in sched.counts:
            sems[space] = semctx.enter_context(nc.semaphore(f"sem_{space}"))

        with nc.Block() as block:
            def runner(engine_name):
                def run(eng):
                    for fn, waits, space, inc in sched.ops[engine_name]:
                        for ps, v in waits:
                            eng.wait_ge(sems[ps], v)
                        inst = fn()
                        inst.then_inc(sems[space], inc)
                return run

            block.vector(runner("vector"))
            block.scalar(runner("scalar"))
            block.gpsimd(runner("gpsimd"))
            block.sync(runner("sync"))
        ctx.close()
    return nc


def _emit_chunk(nc, sched, tl, ch, par, F, xv, ov, chains, c2, affine, kb_t,
                nbias, mybir):
    dt = mybir.dt
    AO = mybir.AluOpType
    AF = mybir.ActivationFunctionType
    V, SC, G = nc.vector, nc.scalar, nc.gpsimd
    S = sched.add
    k = lambda name: f"{name}{par}"      # tile keys per buffer parity

    bufA, s4, c4, s2 = tl["bufA"], tl["s4"], tl["c4"], tl["s2"]
    sincm, coscm, st, w_t = tl["sincm"], tl["coscm"], tl["st"], tl["w_t"]
    scrA, scrB, uvt, tot, rr = tl["scrA"], tl["scrB"], tl["uvt"], tl["tot"], tl["rr"]
    st2t16, sq32, out32 = tl["st2t"], tl["sq32"], tl["out32"]
    xr = bufA

    # ---------- DMA in (two halves so trig overlaps the transfer)
    H = F // 2
    S("sync", lambda: nc.sync.dma_start(xr[:, 0:12 * H],
                                        xv[:, ch * 12 * F:ch * 12 * F + 12 * H]),
      writes=[k("bufA") + "h0"], space=f"dma_in_{par}", inc=16)
    S("sync", lambda: nc.sync.dma_start(xr[:, 12 * H:12 * F],
                                        xv[:, ch * 12 * F + 12 * H:(ch + 1) * 12 * F]),
      writes=[k("bufA") + "h1"], space=f"dma_in_{par}", inc=16)

    # ---------- xs affine (general path)
    trig_key = k("bufA")
    if affine:
        xs = scrA
        kbc = kb_t[:, 0:12].unsqueeze(1).broadcast_to([P, F, 12])
        bbc = kb_t[:, 12:24].unsqueeze(1).broadcast_to([P, F, 12])
        x3 = lambda t: t[:, 0:12 * F].rearrange("p (f c) -> p f c", c=12)
        S("vector", lambda: V.tensor_tensor(out=x3(xs), in0=x3(xr), in1=kbc, op=AO.mult),
          reads=[k("bufA") + "h0", k("bufA") + "h1", "kb"], writes=[k("scrA")])
        S("vector", lambda: V.tensor_tensor(out=x3(xs), in0=x3(xs), in1=bbc, op=AO.add),
          reads=["kb"], writes=[k("scrA")])
        trig_src, trig_key = xs, k("scrA")
    else:
        trig_src = xr

    # ---------- trig: s4=sin(x/4) [ACT], s2=sin(x/2) [ACT, |x|<=2pi safe];
    # cos = 1-2*s2^2 ; cos(x/2) = 1-2*s4^2 ; sin = 2*s2*cos(x/2). ACT 2 ops,
    # DVE 3 tensor_tensor (fp16 2x) + 3 tensor_scalar (fp16 4x).
    cm = lambda t, w=12: t[:, 0:w * F].rearrange("p (c f) -> p c f", c=w)
    for h in (0, 1):
        f0, f1 = h * H, (h + 1) * H
        scm = trig_src[:, 12 * f0:12 * f1].rearrange("p (f c) -> p c f", c=12)
        hk = trig_key + f"h{h}" if not affine else trig_key
        S("scalar", lambda scm=scm, f0=f0, f1=f1: SC.activation(
            cm(s4)[:, :, f0:f1], scm, AF.Sin, scale=0.25),
          reads=[hk], writes=[k("s4") + f"h{h}"])
        S("scalar", lambda scm=scm, f0=f0, f1=f1: SC.activation(
            cm(s2)[:, :, f0:f1], scm, AF.Sin, scale=0.5),
          reads=[hk], writes=[k("s2") + f"h{h}"])
    for h in (0, 1):
        f0, f1 = h * H, (h + 1) * H
        hs = lambda t, f0=f0, f1=f1: cm(t)[:, :, f0:f1]
        hk4, hk2 = k("s4") + f"h{h}", k("s2") + f"h{h}"
        hkc4 = k("c4") + f"h{h}"
        S("vector", lambda hs=hs: V.tensor_tensor(out=hs(c4), in0=hs(s4),
                                                  in1=hs(s4), op=AO.mult),
          reads=[hk4], writes=[hkc4])
        S("vector", lambda hs=hs: V.tensor_scalar(out=hs(c4), in0=hs(c4),
                                                  scalar1=-2.0, scalar2=1.0,
                                                  op0=AO.mult, op1=AO.add),
          reads=[hkc4], writes=[hkc4])
        S("vector", lambda hs=hs: V.tensor_tensor(out=hs(coscm), in0=hs(s2),
                                                  in1=hs(s2), op=AO.mult),
          reads=[hk2], writes=[k("coscm")])
        S("vector", lambda hs=hs: V.tensor_scalar(out=hs(coscm), in0=hs(coscm),
                                                  scalar1=-2.0, scalar2=1.0,
                                                  op0=AO.mult, op1=AO.add),
          reads=[k("coscm")], writes=[k("coscm")])
        S("vector", lambda hs=hs: V.tensor_tensor(out=hs(sincm), in0=hs(s2),
                                                  in1=hs(c4), op=AO.mult),
          reads=[hk2, hkc4], writes=[k("sincm")])
        S("vector", lambda hs=hs: V.tensor_scalar(out=hs(sincm), in0=hs(sincm),
                                                  scalar1=2.0, scalar2=None, op0=AO.mult),
          reads=[k("sincm")], writes=[k("sincm")])

    CP1, SP1 = coscm[:, 0:3 * F], sincm[:, 0:3 * F]
    CT1, ST1 = coscm[:, 3 * F:6 * F], sincm[:, 3 * F:6 * F]
    CP2, SP2 = coscm[:, 6 * F:9 * F], sincm[:, 6 * F:9 * F]
    CT2, ST2 = coscm[:, 9 * F:12 * F], sincm[:, 9 * F:12 * F]

    # ---------- features E,F,G,H -> s4 slot (dead after trig)
    efgh = s4
    for idx, (a, b) in enumerate(((CP1, CT1), (SP1, CT1), (CP1, ST1), (SP1, ST1))):
        S("vector", lambda a=a, b=b, idx=idx: V.tensor_tensor(
            out=efgh[:, idx * 3 * F:(idx + 1) * 3 * F], in0=a, in1=b, op=AO.mult),
          reads=[k("sincm"), k("coscm")], writes=[k("s4")])

    def feat_ap(name):
        base = {"E": 0, "F": 1, "G": 2, "H": 3}
        b = int(name[-1])
        if name[0] in base and len(name) == 2:
            i = base[name[0]] * 3 + b
            return efgh[:, i * F:(i + 1) * F]
        if name.startswith("CT1"):
            return coscm[:, (3 + b) * F:(4 + b) * F]
        if name.startswith("ST1"):
            return sincm[:, (3 + b) * F:(4 + b) * F]
        raise KeyError(name)

    def unit_ap(tile, w, m, comp):
        row = (0 if m % 2 == 0 else 2) + comp
        off = row * 6 * F + w * 3 * F + (m // 2) * F
        return tile[:, off:off + F]

    # ---------- V1+C1 chains -> st
    rkeys = [k("s4"), k("sincm"), k("coscm")]
    chain_items = sorted(chains.items(), key=lambda it: it[0][1] % 2)  # T rows first
    ukey = lambda w, m, comp: k("st") + f"x{w}{m}{comp}"
    stT_keys = [ukey(w, m, c_) for (w, m, c_) in chains if m % 2 == 0]
    stB_keys = [ukey(w, m, c_) for (w, m, c_) in chains if m % 2 == 1]
    # first terms / memsets batched first (ACT + gpsimd run ahead of DVE)
    for (w, m, comp), expr in chain_items:
        out_ap = unit_ap(st, w, m, comp)
        stk = ukey(w, m, comp)
        items = list(expr.items())
        if not items:
            S("gpsimd", lambda o=out_ap: G.memset(o, 0.0), writes=[stk])
        else:
            n0, c0 = items[0]
            S("scalar", lambda o=out_ap, n=n0, c=c0: SC.mul(
                o, feat_ap(n), float(c)),
              reads=rkeys, writes=[stk])
    for (w, m, comp), expr in chain_items:
        out_ap = unit_ap(st, w, m, comp)
        stk = ukey(w, m, comp)
        items = list(expr.items())
        # B-rows (m odd) are needed later (V2 rotation); route half to the
        # otherwise-idle Pool engine
        for n, c in items[1:]:
            S("vector", lambda o=out_ap, n=n, c=c: V.scalar_tensor_tensor(
                out=o, in0=feat_ap(n), scalar=float(c), in1=o,
                op0=AO.mult, op1=AO.add),
              reads=rkeys, writes=[stk])

    # ---------- V2 phase: W = ep2 * T
    bc2 = lambda t: t.unsqueeze(1).broadcast_to([P, 2, 3 * F])
    g2 = lambda ap: ap.rearrange("p (g q) -> p g q", g=2)
    TRE, TIM = st[:, 0:6 * F], st[:, 6 * F:12 * F]
    WRE, WIM = w_t[:, 0:6 * F], w_t[:, 6 * F:12 * F]
    vk = stT_keys + [k("coscm"), k("sincm")]
    S("vector", lambda: V.tensor_tensor(out=g2(WRE), in0=bc2(CP2), in1=g2(TRE), op=AO.mult),
      reads=vk, writes=[k("w_t")])
    S("vector", lambda: V.tensor_tensor(out=g2(scrB[:, 0:6 * F]), in0=bc2(SP2),
                                        in1=g2(TIM), op=AO.mult),
      reads=vk, writes=[k("scrB")])
    S("vector", lambda: V.tensor_tensor(out=WRE, in0=WRE, in1=scrB[:, 0:6 * F],
                                        op=AO.subtract),
      reads=[k("scrB")], writes=[k("w_t")])
    S("vector", lambda: V.tensor_tensor(out=g2(WIM), in0=bc2(CP2), in1=g2(TIM), op=AO.mult),
      reads=vk, writes=[k("w_t")])
    S("vector", lambda: V.tensor_tensor(out=g2(scrB[:, 6 * F:12 * F]), in0=bc2(SP2),
                                        in1=g2(TRE), op=AO.mult),
      reads=vk, writes=[k("scrB")])
    S("vector", lambda: V.tensor_tensor(out=WIM, in0=WIM, in1=scrB[:, 6 * F:12 * F],
                                        op=AO.add),
      reads=[k("scrB")], writes=[k("w_t")])

    # ---------- V2 rotation -> st2t (fp16)
    st2t = st2t16
    bc4 = lambda t: t.unsqueeze(1).broadcast_to([P, 4, 3 * F])
    g4 = lambda ap: ap.rearrange("p (g q) -> p g q", g=4)
    Brows = st[:, 12 * F:24 * F]
    Tp, Bp = st2t[:, 0:12 * F], st2t[:, 12 * F:24 * F]
    S("vector", lambda: V.tensor_tensor(out=g4(Tp), in0=bc4(CT2), in1=g4(w_t[:, :]),
                                        op=AO.mult),
      reads=[k("w_t"), k("coscm")], writes=[k("bufA")])
    S("vector", lambda: V.tensor_tensor(out=g4(scrB[:, :]), in0=bc4(ST2), in1=g4(Brows),
                                        op=AO.mult),
      reads=stB_keys + [k("sincm")], writes=[k("scrB")])
    S("vector", lambda: V.tensor_tensor(out=Tp, in0=Tp, in1=scrB[:, :], op=AO.subtract),
      reads=[k("scrB")], writes=[k("bufA")])
    S("vector", lambda: V.tensor_tensor(out=g4(Bp), in0=bc4(ST2), in1=g4(w_t[:, :]),
                                        op=AO.mult),
      reads=[k("w_t"), k("sincm")], writes=[k("bufA")])
    S("vector", lambda: V.tensor_tensor(out=g4(scrA[:, :]), in0=bc4(CT2), in1=g4(Brows),
                                        op=AO.mult),
      reads=stB_keys + [k("coscm")], writes=[k("scrA")])
    S("vector", lambda: V.tensor_tensor(out=Bp, in0=Bp, in1=scrA[:, :], op=AO.add),
      reads=[k("scrA")], writes=[k("bufA")])

    # ---------- C2 -> uvt = [URE|UIM|VRE|VIM]
    st2t_v = st2t[:, :].rearrange("p (r h b q) -> p r h b q", r=4, h=2, b=3)
    uvt_v = uvt[:, :].rearrange("p (h c b q) -> p h c b q", h=2, c=2, b=6)
    uv_src = lambda m, comp: st2t_v[:, (0 if m % 2 == 0 else 2) + comp, :, m // 2, :]
    uv_dst = lambda m, comp: uvt_v[:, :, comp, m, :]

    SC2 = float(np.sqrt(128.0))  # scale u,v by sqrt(128): amp x128 so fp16
    # squares stay normal; normalize cancels the scale exactly.
    c2_units = []
    for (i, j), t, p in c2:
        al = np.exp(1j * p) * np.cos(t) * SC2
        be = np.exp(1j * p) * np.sin(t) * SC2
        c, s = np.cos(t) * SC2, np.sin(t) * SC2
        for comp in (0, 1):
            sgn = -1.0 if comp == 0 else 1.0
            for (tgt, c0, c1_, c2_) in (
                (i, al.real, sgn * al.imag, -s),
                (j, be.real, sgn * be.imag, c),
            ):
                c2_units.append((i, j, comp, tgt, c0, c1_, c2_))
    # first terms batched on ACT (per-unit uvt keys -> no false serialization)
    for (i, j, comp, tgt, c0, c1_, c2_) in c2_units:
        uk = k("uvt") + f"u{tgt}c{comp}"
        S("scalar", lambda tgt=tgt, comp=comp, m=i, c0=c0: SC.mul(
            uv_dst(tgt, comp), uv_src(m, comp), float(c0)),
          reads=[k("bufA")], writes=[uk])
    for un, (i, j, comp, tgt, c0, c1_, c2_) in enumerate(c2_units):
        uk = k("uvt") + f"u{tgt}c{comp}"
        o = uv_dst(tgt, comp)
        S("vector", lambda o=o, m=i, cc=1 - comp, c1_=c1_: V.scalar_tensor_tensor(
            out=o, in0=uv_src(m, cc), scalar=float(c1_), in1=o,
            op0=AO.mult, op1=AO.add),
          reads=[k("bufA")], writes=[uk])
        S("vector", lambda o=o, m=j, cc=comp, c2_=c2_: V.scalar_tensor_tensor(
            out=o, in0=uv_src(m, cc), scalar=float(c2_), in1=o,
            op0=AO.mult, op1=AO.add),
          reads=[k("bufA")], writes=[uk])

    uvt_keys = [k("uvt")] + [k("uvt") + f"u{m}c{c_}" for m in (1, 2, 3, 4)
                             for c_ in (0, 1)]
    # modes 0,5 pass-through copies
    for w in (0, 1):
        for comp in (0, 1):
            src_off = comp * 6 * F + w * 3 * F
            d0 = uvt[:, w * 12 * F + comp * 6 * F:w * 12 * F + comp * 6 * F + F]
            d5 = uvt[:, w * 12 * F + comp * 6 * F + 5 * F:
                     w * 12 * F + comp * 6 * F + 6 * F]
            S("scalar", lambda d=d0, o=src_off: SC.mul(d, st2t[:, o:o + F], SC2),
              reads=[k("bufA")], writes=[k("uvt")])
            S("scalar", lambda d=d5, o=src_off: SC.mul(d, st2t[:, o + 14 * F:o + 15 * F], SC2),
              reads=[k("bufA")], writes=[k("uvt")])

    # ---------- amplitudes (d-major pair order)
    URE, UIM = uvt[:, 0:6 * F], uvt[:, 6 * F:12 * F]
    VRE, VIM = uvt[:, 12 * F:18 * F], uvt[:, 18 * F:24 * F]
    UU = uvt[:, 0:12 * F].rearrange("p (c q) -> p c q", c=2)
    VV = uvt[:, 12 * F:24 * F].rearrange("p (c q) -> p c q", c=2)
    amp_re, amp_im = c4, s2
    # two-pass amp: all products first (Pool t12 || DVE t34+im), then all
    # combines -- no in-order stall of DVE combines behind Pool products.
    # Scratch regions carved from dead fp16 tiles; safety: every product op
    # waits on uvt_keys, whose writers (C2, vector+scalar) postdate all prior
    # readers of these tiles on their engines.
    freelist = [[st, 0, 24], [st2t16, 0, 24], [scrA, 0, 12], [scrB, 0, 12],
                [w_t, 0, 12], [coscm, 0, 12], [s4, 0, 12], [sincm, 0, 15]]
    def alloc(nF):
        for ent in freelist:
            t, off, cap = ent
            if cap - off >= nF:
                ent[1] = off + nF
                return t[:, off * F:(off + nF) * F]
        raise RuntimeError("amp scratch oom")
    regs = {}
    for d in range(1, 6):
        w = (6 - d) * F
        wf = 6 - d
        regs[d] = dict(t12=alloc(2 * wf), t34=alloc(2 * wf),
                       im=[alloc(wf) for _ in range(4)])
    rk = lambda d, nm: k("ampr") + f"{d}{nm}"
    for d in range(1, 6):
        w = (6 - d) * F
        r = regs[d]
        t12v = r["t12"].rearrange("p (c q) -> p c q", c=2)
        t34v = r["t34"].rearrange("p (c q) -> p c q", c=2)
        S("gpsimd", lambda t12v=t12v, w=w, d=d: G.tensor_tensor(
            out=t12v, in0=UU[:, :, 0:w], in1=VV[:, :, d * F:d * F + w], op=AO.mult),
          reads=uvt_keys, writes=[rk(d, "t12")])
        S("vector", lambda t34v=t34v, w=w, d=d: V.tensor_tensor(
            out=t34v, in0=UU[:, :, d * F:d * F + w], in1=VV[:, :, 0:w], op=AO.mult),
          reads=uvt_keys, writes=[rk(d, "t34")])
        for ii, (ua, va, o1, o2) in enumerate((
                (URE, VIM, 0, 1), (UIM, VRE, 0, 1),
                (URE, VIM, 1, 0), (UIM, VRE, 1, 0))):
            en, enm = (G, "gpsimd") if ii == 0 else (V, "vector")
            S(enm, lambda w=w, d=d, im=r["im"][ii], ua=ua, va=va,
              o1=o1, o2=o2, e=en: e.tensor_tensor(
                out=im[:, 0:w], in0=ua[:, o1 * d * F:o1 * d * F + w],
                in1=va[:, o2 * d * F:o2 * d * F + w], op=AO.mult),
              reads=uvt_keys, writes=[rk(d, f"im{ii}")])
    doff = 0
    for d in range(1, 6):
        w = (6 - d) * F
        r = regs[d]
        are = amp_re[:, doff:doff + w]
        aim = amp_im[:, doff:doff + w]
        S("vector", lambda are=are, w=w, t12=r["t12"]: V.tensor_tensor(
            out=are, in0=t12[:, 0:w], in1=t12[:, w:2 * w], op=AO.subtract),
          reads=[rk(d, "t12")], writes=[k("c4")])
        S("vector", lambda are=are, w=w, t34=r["t34"]: V.tensor_tensor(
            out=are, in0=are, in1=t34[:, 0:w], op=AO.add),
          reads=[rk(d, "t34")], writes=[k("c4")])
        S("vector", lambda are=are, w=w, t34=r["t34"]: V.tensor_tensor(
            out=are, in0=are, in1=t34[:, w:2 * w], op=AO.subtract),
          reads=[rk(d, "t34")], writes=[k("c4")])
        S("vector", lambda aim=aim, w=w, i0=r["im"][0], i1=r["im"][1]:
          V.tensor_tensor(out=aim, in0=i0[:, 0:w], in1=i1[:, 0:w], op=AO.add),
          reads=[rk(d, "im0"), rk(d, "im1")], writes=[k("s2")])
        S("vector", lambda aim=aim, w=w, i2=r["im"][2]: V.tensor_tensor(
            out=aim, in0=aim, in1=i2[:, 0:w], op=AO.add),
          reads=[rk(d, "im2")], writes=[k("s2")])
        S("vector", lambda aim=aim, w=w, i3=r["im"][3]: V.tensor_tensor(
            out=aim, in0=aim, in1=i3[:, 0:w], op=AO.add),
          reads=[rk(d, "im3")], writes=[k("s2")])
        doff += w

    # ---------- tail (amp scaled x128 so fp16 squares stay normal-range;
    # normalize cancels the scale)
    sqre, sqim = sincm, st  # dead fp16 tiles, [15F] each
    S("vector", lambda: V.tensor_tensor(out=sqre[:, 0:15 * F], in0=amp_re[:, 0:15 * F],
                                        in1=amp_re[:, 0:15 * F], op=AO.mult),
      reads=[k("c4")], writes=[k("sqre"), k("sincm")])
    S("scalar", lambda: SC.activation(sqim[:, 0:15 * F], amp_im[:, 0:15 * F],
                                      AF.Square),
      reads=[k("s2")], writes=[k("sqim")] + stT_keys + stB_keys)
    S("vector", lambda: V.tensor_tensor(out=sqre[:, 0:15 * F], in0=sqre[:, 0:15 * F],
                                        in1=sqim[:, 0:15 * F], op=AO.add),
      reads=[k("sqre"), k("sqim")], writes=[k("sqre")])
    # fold q8..14 into q0..6 (fp16 2x), then reduce the remaining 8 q-slots
    S("vector", lambda: V.tensor_tensor(out=sqim[:, 0:7 * F], in0=sqre[:, 0:7 * F],
                                        in1=sqre[:, 8 * F:15 * F], op=AO.add),
      reads=[k("sqre")], writes=[k("sqim2")])
    S("vector", lambda: V.tensor_scalar(out=sqim[:, 7 * F:8 * F], in0=sqre[:, 14 * F:15 * F],
                                        scalar1=1.0, scalar2=None, op0=AO.mult),
      reads=[k("sqre")], writes=[k("sqim2")])
    S("vector", lambda: V.tensor_tensor(out=sqim[:, 0:4 * F], in0=sqim[:, 0:4 * F],
                                        in1=sqim[:, 4 * F:8 * F], op=AO.add),
      reads=[k("sqim2")], writes=[k("sqim2")])
    S("vector", lambda: V.tensor_tensor(out=sqim[:, 0:2 * F], in0=sqim[:, 0:2 * F],
                                        in1=sqim[:, 2 * F:4 * F], op=AO.add),
      reads=[k("sqim2")], writes=[k("sqim2")])
    S("vector", lambda: V.tensor_tensor(out=tot[:, :], in0=sqim[:, 0:F],
                                        in1=sqim[:, F:2 * F], op=AO.add),
      reads=[k("sqim2")], writes=[k("tot")])
    S("vector", lambda: V.tensor_scalar_max(out=tot[:, :], in0=tot[:, :],
                                            scalar1=float(EPS * EPS)),
      writes=[k("tot")])
    S("scalar", lambda: SC.activation(rr[:, :], tot[:, :], AF.Sqrt),
      reads=[k("tot")], writes=[k("rr")])
    S("vector", lambda: V.reciprocal(out=rr[:, :], in_=rr[:, :]), writes=[k("rr")])
    rr16 = w_t  # dead fp16 [12F]; use [0:F]
    S("vector", lambda: V.tensor_scalar(out=rr16[:, 0:F], in0=rr[:, 0:F],
                                        scalar1=1.0, scalar2=None, op0=AO.mult),
      reads=[k("rr")], writes=[k("rr16")])
    root = c4     # amp_re source; sqrt overwrites after squares done
    out16 = st2t16  # dead fp16 [24F]
    cmaj = lambda t: t[:, 0:15 * F].rearrange("p (q f) -> p q f", q=15)
    # split sqrt/scale/DMA by q-ranges: output DMA slices stay contiguous
    # (4KB runs) instead of 256B strided f-halves
    for h, (q0, q1) in enumerate(((0, 8), (8, 15))):
        S("scalar", lambda q0=q0, q1=q1: SC.activation(
            cmaj(root)[:, q0:q1, :], cmaj(sqre)[:, q0:q1, :], AF.Sqrt),
          reads=[k("sqre")], writes=[k("c4") + f"h{h}"])
        S("vector", lambda q0=q0, q1=q1: V.tensor_tensor(
            out=cmaj(out16)[:, q0:q1, :],
            in0=cmaj(root)[:, q0:q1, :],
            in1=rr16[:, 0:F].unsqueeze(1).broadcast_to([P, q1 - q0, F]),
            op=AO.mult),
          reads=[k("c4") + f"h{h}", k("rr16")],
          writes=[k("out16") + f"o{h}"] + stT_keys + stB_keys)
        S("sync", lambda q0=q0, q1=q1: nc.sync.dma_start(
            ov[:, 15 * F * ch + q0 * F:15 * F * ch + q1 * F],
            out16[:, q0 * F:q1 * F]),
          reads=[k("out16") + f"o{h}"], space=f"dma_out_{par}", inc=16)


def kernel(x, param_phi, param_theta, input_k, input_b):
    from concourse.bass_utils import run_bass_kernel_spmd

    x = np.ascontiguousarray(np.asarray(x, np.float16))
    key = (tuple(np.asarray(param_phi, np.float64).tolist()),
           tuple(np.asarray(param_theta, np.float64).tolist()),
           tuple(np.asarray(input_k, np.float64).tolist()),
           tuple(np.asarray(input_b, np.float64).tolist()))
    if key not in _CACHE:
        _CACHE[key] = _build(param_phi, param_theta, input_k, input_b)
    nc = _CACHE[key]

    kv = np.asarray(input_k, np.float64)
    bv = np.asarray(input_b, np.float64)
    affine = not (np.allclose(kv, 1.0) and np.allclose(bv, 0.0))

    in_maps = []
    for c in range(NCORES):
        m = {"x": x[c * COREB:(c + 1) * COREB]}
        if affine:
            kb = np.concatenate([kv, bv]).astype(np.float32)[None, :].repeat(P, 0)
            m["kb"] = np.ascontiguousarray(kb)
        in_maps.append(m)

    res = run_bass_kernel_spmd(nc, in_maps, core_ids=list(range(NCORES)))
    # device out: [P, 15*FTOT] fp16, layout [p, q*F + f] (q = d-major pair idx)
    devs = []
    for r in res.results:
        d = np.asarray(r["out"], np.float32).reshape(P, 15, FTOT)
        devs.append(d.transpose(0, 2, 1).reshape(COREB, 15))
    dev = np.concatenate(devs, axis=0)
    out = np.empty_like(dev)
    for dpos, pair in enumerate(DPAIRS):
        out[:, OUT_PAIRS.index(pair)] = dev[:, dpos]
    return out



# revision 72
# speedup vs baseline: 1.0103x; 1.0077x over previous
"""Trainium2 Bass kernel for the BQNN boson-sampling MZI circuit (raw Bass).

Per sample: 6x6 unitary from 14 MZI Givens blocks applied to e0,e3 -> u,v;
out = |normalize(amp)|, amp_ab = u_a v_b + u_b v_a over 15 pairs.

Host-folded structure: const steps 0-3 -> constant real u0,v0; V1(+C1) on
those constants collapses to short zero-pruned linear chains over per-block
trig features; V2 is a packed generic layer; C2 emits straight into the
amp-stage layout.  sin/cos computed via quarter-angle double-angle (ACT Sin
is only valid on [-pi,pi]).  Raw Bass + explicit semaphore scoreboard
(TileContext tail-drain is rejected by this walrus build).

Layout: per core 32768 samples = 128 partitions x (n_chunks x F) free.
"""

import contextlib
import numpy as np

P = 128
NCORES = 8
BATCH = 262144
COREB = BATCH // NCORES        # 32768
FTOT = COREB // P              # 256

MODES = [[0, 1], [4, 5], [1, 2], [3, 4]] + [[0, 1], [2, 3], [4, 5], [1, 2], [3, 4]] * 2
OUT_PAIRS = [(i, j) for i in range(6) for j in range(i + 1, 6)]
DPAIRS = [(a, a + d) for d in range(1, 6) for a in range(6 - d)]
EPS = 1e-12

_CACHE = {}


def _host_consts(param_phi, param_theta):
    th = np.asarray(param_theta, np.float64)
    ph = np.asarray(param_phi, np.float64)
    U = np.eye(6, dtype=np.complex128)
    for k in range(4):
        i, j = MODES[k]
        c, s = np.cos(th[k]), np.sin(th[k])
        ri, rj = U[i, :].copy(), U[j, :].copy()
        U[i, :] = c * ri - s * rj
        U[j, :] = s * ri + c * rj
    u0, v0 = U[:, 0].copy(), U[:, 3].copy()
    c1 = [(MODES[7], th[4], ph[0]), (MODES[8], th[5], ph[1])]
    c2 = [(MODES[12], th[6], ph[2]), (MODES[13], th[7], ph[3])]
    return u0, v0, c1, c2


def _v1c1_exprs(u0, v0, c1):
    exprs = {}
    for w, w0 in ((0, u0), (1, v0)):
        for b in range(3):
            at, ab = w0[2 * b], w0[2 * b + 1]
            E, F_, G, H = f"E{b}", f"F{b}", f"G{b}", f"H{b}"
            CT, ST = f"CT1{b}", f"ST1{b}"
            exprs[(w, 2 * b, 0)] = {E: at.real, F_: -at.imag, ST: -ab.real}
            exprs[(w, 2 * b, 1)] = {E: at.imag, F_: at.real, ST: -ab.imag}
            exprs[(w, 2 * b + 1, 0)] = {G: at.real, H: -at.imag, CT: ab.real}
            exprs[(w, 2 * b + 1, 1)] = {G: at.imag, H: at.real, CT: ab.imag}

    def comb(*terms):
        out = {}
        for coef, d in terms:
            for k, v in d.items():
                out[k] = out.get(k, 0.0) + coef * v
        return out

    for (i, j), t, p in c1:
        al = np.exp(1j * p) * np.cos(t)
        be = np.exp(1j * p) * np.sin(t)
        c, s = np.cos(t), np.sin(t)
        for w in (0, 1):
            zir, zii = exprs[(w, i, 0)], exprs[(w, i, 1)]
            zjr, zji = exprs[(w, j, 0)], exprs[(w, j, 1)]
            exprs[(w, i, 0)] = comb((al.real, zir), (-al.imag, zii), (-s, zjr))
            exprs[(w, i, 1)] = comb((al.real, zii), (al.imag, zir), (-s, zji))
            exprs[(w, j, 0)] = comb((be.real, zir), (-be.imag, zii), (c, zjr))
            exprs[(w, j, 1)] = comb((be.real, zii), (be.imag, zir), (c, zji))
    return {k: {n: c for n, c in d.items() if abs(c) > 1e-30} for k, d in exprs.items()}


class Sched:
    """Per-engine in-order op lists + semaphore scoreboard.

    Engines: vector / scalar / gpsimd / sync.  Each op incs a sem space; DMA
    ops (engine sync) inc per-chunk-parity spaces so out-of-order DMA-queue
    completion can't satisfy another chunk's wait.
    """

    def __init__(self):
        self.ops = {"vector": [], "scalar": [], "gpsimd": [], "sync": []}
        self.counts = {}          # sem space -> current value
        self.waited = {"vector": {}, "scalar": {}, "gpsimd": {}, "sync": {}}
        self.writers = {}         # tile key -> [(space, val)]
        self.readers = {}

    def add(self, engine, fn, reads=(), writes=(), space=None, inc=1):
        space = space or engine
        self.counts.setdefault(space, 0)
        need = {}
        for r in list(reads) + list(writes):
            for ps, v in self.writers.get(r, ()):
                if ps != space:
                    need[ps] = max(need.get(ps, 0), v)
        for wkey in writes:
            for ps, v in self.readers.get(wkey, ()):
                if ps != space:
                    need[ps] = max(need.get(ps, 0), v)
        waits = []
        wt = self.waited[engine]
        for ps, v in sorted(need.items()):
            if wt.get(ps, 0) < v:
                waits.append((ps, v))
                wt[ps] = v
        after = self.counts[space] + inc
        self.counts[space] = after
        for r in reads:
            self.readers.setdefault(r, []).append((space, after))
        for wkey in writes:
            self.writers.setdefault(wkey, []).append((space, after))
        self.ops[engine].append((fn, waits, space, inc))


def _build(param_phi, param_theta, input_k, input_b, n_chunks=1):
    import concourse.bass as bass
    import concourse.mybir as mybir

    dt = mybir.dt
    f32 = dt.float32
    f16 = dt.float16
    AO = mybir.AluOpType
    AF = mybir.ActivationFunctionType

    F = FTOT // n_chunks
    u0, v0, c1, c2 = _host_consts(param_phi, param_theta)
    chains = _v1c1_exprs(u0, v0, c1)

    kv = np.asarray(input_k, np.float64)
    bv = np.asarray(input_b, np.float64)
    affine = not (np.allclose(kv, 1.0) and np.allclose(bv, 0.0))

    nc = bass.Bass()
    x_d = nc.dram_tensor("x", [COREB, 12], f16, kind="ExternalInput")
    o_d = nc.dram_tensor("out", [P, 15 * FTOT], f16, kind="ExternalOutput")
    if affine:
        kb_d = nc.dram_tensor("kb", [P, 24], f32, kind="ExternalInput")
    xv = x_d.rearrange("(p f) c -> p (f c)", p=P)
    ov = o_d

    ctx = contextlib.ExitStack()
    sb = lambda nm, w, dty=f32: ctx.enter_context(nc.sbuf_tensor(nm, [P, w], dty))
    npar = min(n_chunks, 2)
    tiles = []
    # fp16 tiles: all intermediate math (2x DVE tensor_tensor, 4x tensor_scalar)
    widths16 = dict(s4=12, c4=15, s2=15, sincm=15, coscm=12, st=24,
                    w_t=12, scrA=12, scrB=12, uvt=24, st2t=24)
    for par in range(npar):
        tl = {nm: sb(f"{nm}_{par}", w * F, f16) for nm, w in widths16.items()}
        tl["bufA"] = sb(f"bufA_{par}", 12 * F, f16)  # fp16 DMA-in x
        tl["sq32"] = sb(f"sq32_{par}", 30 * F)      # f32 squares (underflow-safe)
        tl["out32"] = sb(f"out32_{par}", 15 * F)    # f32 output staging
        tl["tot"] = sb(f"tot_{par}", F)
        tl["rr"] = sb(f"rr_{par}", F)
        tiles.append(tl)
    nbias = sb("nbias", 1)
    kb_t = sb("kbt", 24) if affine else None

    sched = Sched()
    S = sched.add

    if affine:
        S("sync", lambda: nc.sync.dma_start(kb_t[:, :], kb_d[:, :]),
          writes=["kb"], space="dma_in_0", inc=16)

    for ch in range(n_chunks):
        _emit_chunk(nc, sched, tiles[ch % npar], ch, ch % npar, F, xv, ov,
                    chains, c2, affine, kb_t, nbias, mybir)

    sems = {}
    with contextlib.ExitStack() as semctx:
        for space in sched.counts:
            sems[space] = semctx.enter_context(nc.semaphore(f"sem_{space}"))

        with nc.Block() as block:
            def runner(engine_name):
                def run(eng):
                    for fn, waits, space, inc in sched.ops[engine_name]:
                        for ps, v in waits:
                            eng.wait_ge(sems[ps], v)
                        inst = fn()
                        inst.then_inc(sems[space], inc)
                return run

            block.vector(runner("vector"))
            block.scalar(runner("scalar"))
            block.gpsimd(runner("gpsimd"))
            block.sync(runner("sync"))
        ctx.close()
    return nc


def _emit_chunk(nc, sched, tl, ch, par, F, xv, ov, chains, c2, affine, kb_t,
                nbias, mybir):
    dt = mybir.dt
    AO = mybir.AluOpType
    AF = mybir.ActivationFunctionType
    V, SC, G = nc.vector, nc.scalar, nc.gpsimd
    S = sched.add
    k = lambda name: f"{name}{par}"      # tile keys per buffer parity

    bufA, s4, c4, s2 = tl["bufA"], tl["s4"], tl["c4"], tl["s2"]
    sincm, coscm, st, w_t = tl["sincm"], tl["coscm"], tl["st"], tl["w_t"]
    scrA, scrB, uvt, tot, rr = tl["scrA"], tl["scrB"], tl["uvt"], tl["tot"], tl["rr"]
    st2t16, sq32, out32 = tl["st2t"], tl["sq32"], tl["out32"]
    xr = bufA

    # ---------- DMA in (two halves so trig overlaps the transfer)
    H = F // 2
    S("sync", lambda: nc.sync.dma_start(xr[:, 0:12 * H],
                                        xv[:, ch * 12 * F:ch * 12 * F + 12 * H]),
      writes=[k("bufA") + "h0"], space=f"dma_in_{par}", inc=16)
    S("sync", lambda: nc.sync.dma_start(xr[:, 12 * H:12 * F],
                                        xv[:, ch * 12 * F + 12 * H:(ch + 1) * 12 * F]),
      writes=[k("bufA") + "h1"], space=f"dma_in_{par}", inc=16)

    # ---------- xs affine (general path)
    trig_key = k("bufA")
    if affine:
        xs = scrA
        kbc = kb_t[:, 0:12].unsqueeze(1).broadcast_to([P, F, 12])
        bbc = kb_t[:, 12:24].unsqueeze(1).broadcast_to([P, F, 12])
        x3 = lambda t: t[:, 0:12 * F].rearrange("p (f c) -> p f c", c=12)
        S("vector", lambda: V.tensor_tensor(out=x3(xs), in0=x3(xr), in1=kbc, op=AO.mult),
          reads=[k("bufA") + "h0", k("bufA") + "h1", "kb"], writes=[k("scrA")])
        S("vector", lambda: V.tensor_tensor(out=x3(xs), in0=x3(xs), in1=bbc, op=AO.add),
          reads=["kb"], writes=[k("scrA")])
        trig_src, trig_key = xs, k("scrA")
    else:
        trig_src = xr

    # ---------- trig: s4=sin(x/4) [ACT], s2=sin(x/2) [ACT, |x|<=2pi safe];
    # cos = 1-2*s2^2 ; cos(x/2) = 1-2*s4^2 ; sin = 2*s2*cos(x/2). ACT 2 ops,
    # DVE 3 tensor_tensor (fp16 2x) + 3 tensor_scalar (fp16 4x).
    cm = lambda t, w=12: t[:, 0:w * F].rearrange("p (c f) -> p c f", c=w)
    for h in (0, 1):
        f0, f1 = h * H, (h + 1) * H
        scm = trig_src[:, 12 * f0:12 * f1].rearrange("p (f c) -> p c f", c=12)
        hk = trig_key + f"h{h}" if not affine else trig_key
        S("scalar", lambda scm=scm, f0=f0, f1=f1: SC.activation(
            cm(s4)[:, :, f0:f1], scm, AF.Sin, scale=0.25),
          reads=[hk], writes=[k("s4") + f"h{h}"])
        S("scalar", lambda scm=scm, f0=f0, f1=f1: SC.activation(
            cm(s2)[:, :, f0:f1], scm, AF.Sin, scale=0.5),
          reads=[hk], writes=[k("s2") + f"h{h}"])
    for h in (0, 1):
        f0, f1 = h * H, (h + 1) * H
        hs = lambda t, f0=f0, f1=f1: cm(t)[:, :, f0:f1]
        hk4, hk2 = k("s4") + f"h{h}", k("s2") + f"h{h}"
        hkc4 = k("c4") + f"h{h}"
        S("vector", lambda hs=hs: V.tensor_tensor(out=hs(c4), in0=hs(s4),
                                                  in1=hs(s4), op=AO.mult),
          reads=[hk4], writes=[hkc4])
        S("vector", lambda hs=hs: V.tensor_scalar(out=hs(c4), in0=hs(c4),
                                                  scalar1=-2.0, scalar2=1.0,
                                                  op0=AO.mult, op1=AO.add),
          reads=[hkc4], writes=[hkc4])
        S("vector", lambda hs=hs: V.tensor_tensor(out=hs(coscm), in0=hs(s2),
                                                  in1=hs(s2), op=AO.mult),
          reads=[hk2], writes=[k("coscm")])
        S("vector", lambda hs=hs: V.tensor_scalar(out=hs(coscm), in0=hs(coscm),
                                                  scalar1=-2.0, scalar2=1.0,
                                                  op0=AO.mult, op1=AO.add),
          reads=[k("coscm")], writes=[k("coscm")])
        S("vector", lambda hs=hs: V.tensor_tensor(out=hs(sincm), in0=hs(s2),
                                                  in1=hs(c4), op=AO.mult),
          reads=[hk2, hkc4], writes=[k("sincm")])
        S("vector", lambda hs=hs: V.tensor_scalar(out=hs(sincm), in0=hs(sincm),
                                                  scalar1=2.0, scalar2=None, op0=AO.mult),
          reads=[k("sincm")], writes=[k("sincm")])

    CP1, SP1 = coscm[:, 0:3 * F], sincm[:, 0:3 * F]
    CT1, ST1 = coscm[:, 3 * F:6 * F], sincm[:, 3 * F:6 * F]
    CP2, SP2 = coscm[:, 6 * F:9 * F], sincm[:, 6 * F:9 * F]
    CT2, ST2 = coscm[:, 9 * F:12 * F], sincm[:, 9 * F:12 * F]

    # ---------- features E,F,G,H -> s4 slot (dead after trig)
    efgh = s4
    for idx, (a, b) in enumerate(((CP1, CT1), (SP1, CT1), (CP1, ST1), (SP1, ST1))):
        S("vector", lambda a=a, b=b, idx=idx: V.tensor_tensor(
            out=efgh[:, idx * 3 * F:(idx + 1) * 3 * F], in0=a, in1=b, op=AO.mult),
          reads=[k("sincm"), k("coscm")], writes=[k("s4")])

    def feat_ap(name):
        base = {"E": 0, "F": 1, "G": 2, "H": 3}
        b = int(name[-1])
        if name[0] in base and len(name) == 2:
            i = base[name[0]] * 3 + b
            return efgh[:, i * F:(i + 1) * F]
        if name.startswith("CT1"):
            return coscm[:, (3 + b) * F:(4 + b) * F]
        if name.startswith("ST1"):
            return sincm[:, (3 + b) * F:(4 + b) * F]
        raise KeyError(name)

    def unit_ap(tile, w, m, comp):
        row = (0 if m % 2 == 0 else 2) + comp
        off = row * 6 * F + w * 3 * F + (m // 2) * F
        return tile[:, off:off + F]

    # ---------- V1+C1 chains -> st
    rkeys = [k("s4"), k("sincm"), k("coscm")]
    chain_items = sorted(chains.items(), key=lambda it: it[0][1] % 2)  # T rows first
    ukey = lambda w, m, comp: k("st") + f"x{w}{m}{comp}"
    stT_keys = [ukey(w, m, c_) for (w, m, c_) in chains if m % 2 == 0]
    stB_keys = [ukey(w, m, c_) for (w, m, c_) in chains if m % 2 == 1]
    # first terms / memsets batched first (ACT + gpsimd run ahead of DVE)
    for (w, m, comp), expr in chain_items:
        out_ap = unit_ap(st, w, m, comp)
        stk = ukey(w, m, comp)
        items = list(expr.items())
        if not items:
            S("gpsimd", lambda o=out_ap: G.memset(o, 0.0), writes=[stk])
        else:
            n0, c0 = items[0]
            S("scalar", lambda o=out_ap, n=n0, c=c0: SC.mul(
                o, feat_ap(n), float(c)),
              reads=rkeys, writes=[stk])
    for (w, m, comp), expr in chain_items:
        out_ap = unit_ap(st, w, m, comp)
        stk = ukey(w, m, comp)
        items = list(expr.items())
        # B-rows (m odd) are needed later (V2 rotation); route half to the
        # otherwise-idle Pool engine
        for n, c in items[1:]:
            S("vector", lambda o=out_ap, n=n, c=c: V.scalar_tensor_tensor(
                out=o, in0=feat_ap(n), scalar=float(c), in1=o,
                op0=AO.mult, op1=AO.add),
              reads=rkeys, writes=[stk])

    # ---------- V2 phase: W = ep2 * T
    bc2 = lambda t: t.unsqueeze(1).broadcast_to([P, 2, 3 * F])
    g2 = lambda ap: ap.rearrange("p (g q) -> p g q", g=2)
    TRE, TIM = st[:, 0:6 * F], st[:, 6 * F:12 * F]
    WRE, WIM = w_t[:, 0:6 * F], w_t[:, 6 * F:12 * F]
    vk = stT_keys + [k("coscm"), k("sincm")]
    S("vector", lambda: V.tensor_tensor(out=g2(WRE), in0=bc2(CP2), in1=g2(TRE), op=AO.mult),
      reads=vk, writes=[k("w_t")])
    S("vector", lambda: V.tensor_tensor(out=g2(scrB[:, 0:6 * F]), in0=bc2(SP2),
                                        in1=g2(TIM), op=AO.mult),
      reads=vk, writes=[k("scrB")])
    S("vector", lambda: V.tensor_tensor(out=WRE, in0=WRE, in1=scrB[:, 0:6 * F],
                                        op=AO.subtract),
      reads=[k("scrB")], writes=[k("w_t")])
    S("vector", lambda: V.tensor_tensor(out=g2(WIM), in0=bc2(CP2), in1=g2(TIM), op=AO.mult),
      reads=vk, writes=[k("w_t")])
    S("vector", lambda: V.tensor_tensor(out=g2(scrB[:, 6 * F:12 * F]), in0=bc2(SP2),
                                        in1=g2(TRE), op=AO.mult),
      reads=vk, writes=[k("scrB")])
    S("vector", lambda: V.tensor_tensor(out=WIM, in0=WIM, in1=scrB[:, 6 * F:12 * F],
                                        op=AO.add),
      reads=[k("scrB")], writes=[k("w_t")])

    # ---------- V2 rotation -> st2t (fp16)
    st2t = st2t16
    bc4 = lambda t: t.unsqueeze(1).broadcast_to([P, 4, 3 * F])
    g4 = lambda ap: ap.rearrange("p (g q) -> p g q", g=4)
    Brows = st[:, 12 * F:24 * F]
    Tp, Bp = st2t[:, 0:12 * F], st2t[:, 12 * F:24 * F]
    S("vector", lambda: V.tensor_tensor(out=g4(Tp), in0=bc4(CT2), in1=g4(w_t[:, :]),
                                        op=AO.mult),
      reads=[k("w_t"), k("coscm")], writes=[k("bufA")])
    S("vector", lambda: V.tensor_tensor(out=g4(scrB[:, :]), in0=bc4(ST2), in1=g4(Brows),
                                        op=AO.mult),
      reads=stB_keys + [k("sincm")], writes=[k("scrB")])
    S("vector", lambda: V.tensor_tensor(out=Tp, in0=Tp, in1=scrB[:, :], op=AO.subtract),
      reads=[k("scrB")], writes=[k("bufA")])
    S("vector", lambda: V.tensor_tensor(out=g4(Bp), in0=bc4(ST2), in1=g4(w_t[:, :]),
                                        op=AO.mult),
      reads=[k("w_t"), k("sincm")], writes=[k("bufA")])
    S("vector", lambda: V.tensor_tensor(out=g4(scrA[:, :]), in0=bc4(CT2), in1=g4(Brows),
                                        op=AO.mult),
      reads=stB_keys + [k("coscm")], writes=[k("scrA")])
    S("vector", lambda: V.tensor_tensor(out=Bp, in0=Bp, in1=scrA[:, :], op=AO.add),
      reads=[k("scrA")], writes=[k("bufA")])

    # ---------- C2 -> uvt = [URE|UIM|VRE|VIM]
    st2t_v = st2t[:, :].rearrange("p (r h b q) -> p r h b q", r=4, h=2, b=3)
    uvt_v = uvt[:, :].rearrange("p (h c b q) -> p h c b q", h=2, c=2, b=6)
    uv_src = lambda m, comp: st2t_v[:, (0 if m % 2 == 0 else 2) + comp, :, m // 2, :]
    uv_dst = lambda m, comp: uvt_v[:, :, comp, m, :]

    SC2 = float(np.sqrt(128.0))  # scale u,v by sqrt(128): amp x128 so fp16
    # squares stay normal; normalize cancels the scale exactly.
    c2_units = []
    for (i, j), t, p in c2:
        al = np.exp(1j * p) * np.cos(t) * SC2
        be = np.exp(1j * p) * np.sin(t) * SC2
        c, s = np.cos(t) * SC2, np.sin(t) * SC2
        for comp in (0, 1):
            sgn = -1.0 if comp == 0 else 1.0
            for (tgt, c0, c1_, c2_) in (
                (i, al.real, sgn * al.imag, -s),
                (j, be.real, sgn * be.imag, c),
            ):
                c2_units.append((i, j, comp, tgt, c0, c1_, c2_))
    # first terms batched on ACT (per-unit uvt keys -> no false serialization)
    for (i, j, comp, tgt, c0, c1_, c2_) in c2_units:
        uk = k("uvt") + f"u{tgt}c{comp}"
        S("scalar", lambda tgt=tgt, comp=comp, m=i, c0=c0: SC.mul(
            uv_dst(tgt, comp), uv_src(m, comp), float(c0)),
          reads=[k("bufA")], writes=[uk])
    for un, (i, j, comp, tgt, c0, c1_, c2_) in enumerate(c2_units):
        uk = k("uvt") + f"u{tgt}c{comp}"
        o = uv_dst(tgt, comp)
        S("vector", lambda o=o, m=i, cc=1 - comp, c1_=c1_: V.scalar_tensor_tensor(
            out=o, in0=uv_src(m, cc), scalar=float(c1_), in1=o,
            op0=AO.mult, op1=AO.add),
          reads=[k("bufA")], writes=[uk])
        S("vector", lambda o=o, m=j, cc=comp, c2_=c2_: V.scalar_tensor_tensor(
            out=o, in0=uv_src(m, cc), scalar=float(c2_), in1=o,
            op0=AO.mult, op1=AO.add),
          reads=[k("bufA")], writes=[uk])

    uvt_keys = [k("uvt")] + [k("uvt") + f"u{m}c{c_}" for m in (1, 2, 3, 4)
                             for c_ in (0, 1)]
    # modes 0,5 pass-through copies
    for w in (0, 1):
        for comp in (0, 1):
            src_off = comp * 6 * F + w * 3 * F
            d0 = uvt[:, w * 12 * F + comp * 6 * F:w * 12 * F + comp * 6 * F + F]
            d5 = uvt[:, w * 12 * F + comp * 6 * F + 5 * F:
                     w * 12 * F + comp * 6 * F + 6 * F]
            S("scalar", lambda d=d0, o=src_off: SC.mul(d, st2t[:, o:o + F], SC2),
              reads=[k("bufA")], writes=[k("uvt")])
            S("scalar", lambda d=d5, o=src_off: SC.mul(d, st2t[:, o + 14 * F:o + 15 * F], SC2),
              reads=[k("bufA")], writes=[k("uvt")])

    # ---------- amplitudes (d-major pair order)
    URE, UIM = uvt[:, 0:6 * F], uvt[:, 6 * F:12 * F]
    VRE, VIM = uvt[:, 12 * F:18 * F], uvt[:, 18 * F:24 * F]
    UU = uvt[:, 0:12 * F].rearrange("p (c q) -> p c q", c=2)
    VV = uvt[:, 12 * F:24 * F].rearrange("p (c q) -> p c q", c=2)
    amp_re, amp_im = c4, s2
    # two-pass amp: all products first (Pool t12 || DVE t34+im), then all
    # combines -- no in-order stall of DVE combines behind Pool products.
    # Scratch regions carved from dead fp16 tiles; safety: every product op
    # waits on uvt_keys, whose writers (C2, vector+scalar) postdate all prior
    # readers of these tiles on their engines.
    freelist = [[st, 0, 24], [st2t16, 0, 24], [scrA, 0, 12], [scrB, 0, 12],
                [w_t, 0, 12], [coscm, 0, 12], [s4, 0, 12], [sincm, 0, 15]]
    def alloc(nF):
        for ent in freelist:
            t, off, cap = ent
            if cap - off >= nF:
                ent[1] = off + nF
                return t[:, off * F:(off + nF) * F]
        raise RuntimeError("amp scratch oom")
    regs = {}
    for d in range(1, 6):
        w = (6 - d) * F
        wf = 6 - d
        regs[d] = dict(t12=alloc(2 * wf), t34=alloc(2 * wf),
                       im=[alloc(wf) for _ in range(4)])
    rk = lambda d, nm: k("ampr") + f"{d}{nm}"
    for d in range(1, 6):
        w = (6 - d) * F
        r = regs[d]
        t12v = r["t12"].rearrange("p (c q) -> p c q", c=2)
        t34v = r["t34"].rearrange("p (c q) -> p c q", c=2)
        S("gpsimd", lambda t12v=t12v, w=w, d=d: G.tensor_tensor(
            out=t12v, in0=UU[:, :, 0:w], in1=VV[:, :, d * F:d * F + w], op=AO.mult),
          reads=uvt_keys, writes=[rk(d, "t12")])
        S("vector", lambda t34v=t34v, w=w, d=d: V.tensor_tensor(
            out=t34v, in0=UU[:, :, d * F:d * F + w], in1=VV[:, :, 0:w], op=AO.mult),
          reads=uvt_keys, writes=[rk(d, "t34")])
        for ii, (ua, va, o1, o2) in enumerate((
                (URE, VIM, 0, 1), (UIM, VRE, 0, 1),
                (URE, VIM, 1, 0), (UIM, VRE, 1, 0))):
            en, enm = (G, "gpsimd") if ii == 0 else (V, "vector")
            S(enm, lambda w=w, d=d, im=r["im"][ii], ua=ua, va=va,
              o1=o1, o2=o2, e=en: e.tensor_tensor(
                out=im[:, 0:w], in0=ua[:, o1 * d * F:o1 * d * F + w],
                in1=va[:, o2 * d * F:o2 * d * F + w], op=AO.mult),
              reads=uvt_keys, writes=[rk(d, f"im{ii}")])
    doff = 0
    for d in range(1, 6):
        w = (6 - d) * F
        r = regs[d]
        are = amp_re[:, doff:doff + w]
        aim = amp_im[:, doff:doff + w]
        S("vector", lambda are=are, w=w, t12=r["t12"]: V.tensor_tensor(
            out=are, in0=t12[:, 0:w], in1=t12[:, w:2 * w], op=AO.subtract),
          reads=[rk(d, "t12")], writes=[k("c4")])
        S("vector", lambda are=are, w=w, t34=r["t34"]: V.tensor_tensor(
            out=are, in0=are, in1=t34[:, 0:w], op=AO.add),
          reads=[rk(d, "t34")], writes=[k("c4")])
        S("vector", lambda are=are, w=w, t34=r["t34"]: V.tensor_tensor(
            out=are, in0=are, in1=t34[:, w:2 * w], op=AO.subtract),
          reads=[rk(d, "t34")], writes=[k("c4")])
        S("vector", lambda aim=aim, w=w, i0=r["im"][0], i1=r["im"][1]:
          V.tensor_tensor(out=aim, in0=i0[:, 0:w], in1=i1[:, 0:w], op=AO.add),
          reads=[rk(d, "im0"), rk(d, "im1")], writes=[k("s2")])
        S("vector", lambda aim=aim, w=w, i2=r["im"][2]: V.tensor_tensor(
            out=aim, in0=aim, in1=i2[:, 0:w], op=AO.add),
          reads=[rk(d, "im2")], writes=[k("s2")])
        S("vector", lambda aim=aim, w=w, i3=r["im"][3]: V.tensor_tensor(
            out=aim, in0=aim, in1=i3[:, 0:w], op=AO.add),
          reads=[rk(d, "im3")], writes=[k("s2")])
        doff += w

    # ---------- tail (amp scaled x128 so fp16 squares stay normal-range;
    # normalize cancels the scale)
    sqre, sqim = sincm, st  # dead fp16 tiles, [15F] each
    S("vector", lambda: V.tensor_tensor(out=sqre[:, 0:15 * F], in0=amp_re[:, 0:15 * F],
                                        in1=amp_re[:, 0:15 * F], op=AO.mult),
      reads=[k("c4")], writes=[k("sqre"), k("sincm")])
    S("scalar", lambda: SC.activation(sqim[:, 0:15 * F], amp_im[:, 0:15 * F],
                                      AF.Square),
      reads=[k("s2")], writes=[k("sqim")] + stT_keys + stB_keys)
    S("vector", lambda: V.tensor_tensor(out=sqre[:, 0:15 * F], in0=sqre[:, 0:15 * F],
                                        in1=sqim[:, 0:15 * F], op=AO.add),
      reads=[k("sqre"), k("sqim")], writes=[k("sqre")])
    # fold q8..14 into q0..6 (fp16 2x), then reduce the remaining 8 q-slots
    S("vector", lambda: V.tensor_tensor(out=sqim[:, 0:7 * F], in0=sqre[:, 0:7 * F],
                                        in1=sqre[:, 8 * F:15 * F], op=AO.add),
      reads=[k("sqre")], writes=[k("sqim2")])
    S("vector", lambda: V.tensor_scalar(out=sqim[:, 7 * F:8 * F], in0=sqre[:, 14 * F:15 * F],
                                        scalar1=1.0, scalar2=None, op0=AO.mult),
      reads=[k("sqre")], writes=[k("sqim2")])
    S("vector", lambda: V.tensor_tensor(out=sqim[:, 0:4 * F], in0=sqim[:, 0:4 * F],
                                        in1=sqim[:, 4 * F:8 * F], op=AO.add),
      reads=[k("sqim2")], writes=[k("sqim2")])
    S("vector", lambda: V.tensor_tensor(out=sqim[:, 0:2 * F], in0=sqim[:, 0:2 * F],
                                        in1=sqim[:, 2 * F:4 * F], op=AO.add),
      reads=[k("sqim2")], writes=[k("sqim2")])
    S("vector", lambda: V.tensor_tensor(out=tot[:, :], in0=sqim[:, 0:F],
                                        in1=sqim[:, F:2 * F], op=AO.add),
      reads=[k("sqim2")], writes=[k("tot")])
    S("vector", lambda: V.tensor_scalar_max(out=tot[:, :], in0=tot[:, :],
                                            scalar1=float(EPS * EPS)),
      writes=[k("tot")])
    S("scalar", lambda: SC.activation(rr[:, :], tot[:, :], AF.Sqrt),
      reads=[k("tot")], writes=[k("rr")])
    S("vector", lambda: V.reciprocal(out=rr[:, :], in_=rr[:, :]), writes=[k("rr")])
    rr16 = w_t  # dead fp16 [12F]; use [0:F]
    S("vector", lambda: V.tensor_scalar(out=rr16[:, 0:F], in0=rr[:, 0:F],
                                        scalar1=1.0, scalar2=None, op0=AO.mult),
      reads=[k("rr")], writes=[k("rr16")])
    root = c4     # amp_re source; sqrt overwrites after squares done
    out16 = st2t16  # dead fp16 [24F]
    cmaj = lambda t: t[:, 0:15 * F].rearrange("p (q f) -> p q f", q=15)
    # split sqrt/scale/DMA by q-ranges: output DMA slices stay contiguous
    # (4KB runs) instead of 256B strided f-halves
    for h, (q0, q1) in enumerate(((0, 3), (3, 6), (6, 9), (9, 12), (12, 15))):
        S("scalar", lambda q0=q0, q1=q1: SC.activation(
            cmaj(root)[:, q0:q1, :], cmaj(sqre)[:, q0:q1, :], AF.Sqrt),
          reads=[k("sqre")], writes=[k("c4") + f"h{h}"])
        S("vector", lambda q0=q0, q1=q1: V.tensor_tensor(
            out=cmaj(out16)[:, q0:q1, :],
            in0=cmaj(root)[:, q0:q1, :],
            in1=rr16[:, 0:F].unsqueeze(1).broadcast_to([P, q1 - q0, F]),
            op=AO.mult),
          reads=[k("c4") + f"h{h}", k("rr16")],
          writes=[k("out16") + f"o{h}"] + stT_keys + stB_keys)
        S("sync", lambda q0=q0, q1=q1: nc.sync.dma_start(
            ov[:, 15 * F * ch + q0 * F:15 * F * ch + q1 * F],
            out16[:, q0 * F:q1 * F]),
          reads=[k("out16") + f"o{h}"], space=f"dma_out_{par}", inc=16)


def kernel(x, param_phi, param_theta, input_k, input_b):
    from concourse.bass_utils import run_bass_kernel_spmd

    x = np.ascontiguousarray(np.asarray(x, np.float16))
    key = (tuple(np.asarray(param_phi, np.float64).tolist()),
           tuple(np.asarray(param_theta, np.float64).tolist()),
           tuple(np.asarray(input_k, np.float64).tolist()),
           tuple(np.asarray(input_b, np.float64).tolist()))
    if key not in _CACHE:
        _CACHE[key] = _build(param_phi, param_theta, input_k, input_b)
    nc = _CACHE[key]

    kv = np.asarray(input_k, np.float64)
    bv = np.asarray(input_b, np.float64)
    affine = not (np.allclose(kv, 1.0) and np.allclose(bv, 0.0))

    in_maps = []
    for c in range(NCORES):
        m = {"x": x[c * COREB:(c + 1) * COREB]}
        if affine:
            kb = np.concatenate([kv, bv]).astype(np.float32)[None, :].repeat(P, 0)
            m["kb"] = np.ascontiguousarray(kb)
        in_maps.append(m)

    res = run_bass_kernel_spmd(nc, in_maps, core_ids=list(range(NCORES)))
    # device out: [P, 15*FTOT] fp16, layout [p, q*F + f] (q = d-major pair idx)
    devs = []
    for r in res.results:
        d = np.asarray(r["out"], np.float32).reshape(P, 15, FTOT)
        devs.append(d.transpose(0, 2, 1).reshape(COREB, 15))
    dev = np.concatenate(devs, axis=0)
    out = np.empty_like(dev)
    for dpos, pair in enumerate(DPAIRS):
        out[:, OUT_PAIRS.index(pair)] = dev[:, dpos]
    return out

